# revision 10
# baseline (speedup 1.0000x reference)
"""Trainium2 Bass kernel for nn_AttentionBlock (GroupNorm + 4-head self-attention
over S=4096 + output projection + residual) on x:[2, 256, 64, 64].

Sharding: 8 cores = (batch 2) x (query-chunk 4). Each core receives the full
image of its batch (for GroupNorm stats and K/V over all 4096 positions) plus
its own 1024-query chunk, and produces the exact [256, 1024] output slice.
No cross-core collectives.

v2 structure (per core):
  phase 0: chunked x DMA with bn_stats pipelined right behind each chunk;
           PE does weight transposes + the group-aggregation matmuls early.
           Per-channel scale/shift -> hb (full image) / hq (query chunk), bf16.
  phase 1: K/Q projections per head-pair (col-tiled, PSUM), V projection per
           128-key block; drains split across ScalarE / VectorE.
  phase 2: attention per head-pair. The two heads' score matmuls (K=dk=64)
           run CONCURRENTLY as 64x128 row-tiles (head A on PE rows 0:64,
           head B on rows 64:128 - the kT2/qT2 pair packing puts each head's
           operands on exactly those SBUF partitions). exp splits across
           engines: head A on ScalarE (native EXP), head B on VectorE via a
           Schraudolph bit-trick (y = int16(s*184.665+16250.5) bitcast bf16,
           +-3% per-weight; the softmax ratio cancels the systematic part).
           PV accumulates [65, 2, SQ] in one 4-bank PSUM tile (ones column
           -> denominators). PV(jb-1) is issued between S(jb) and S(jb+1) so
           the PE stays busy during the exp drain.
  phase 3: per pair: reciprocal of the denominator row, DMA partition-
           broadcast (SBUF->SBUF, no DRAM round trip), normalize into
           resT2[p] [128, SQ] (head pair stacked). Out-projection contracts
           both heads of a pair in single K=128 matmuls (owT2 pair-packed),
           accumulating both pairs into one PSUM group; + out_b (with the
           v-bias folded through out_w) + residual, DMA out.
"""

import contextlib
import math

import numpy as np

import concourse.bass as bass
import concourse.tile as tile
from concourse import mybir
from concourse.bacc import Bacc
from concourse.masks import make_identity

# Problem constants (hardcoded per harness contract).
B = 2
C = 256
H = W = 64
S = H * W            # 4096
NH = 4
DK = 64
EPS = 1e-5
SCALE2 = 0.125       # (1/sqrt(sqrt(dk)))^2, folded into Wq/bq
N_CORES = 8
CHUNKS = N_CORES // B    # query chunks per batch
SQ = S // CHUNKS         # queries per core (1024)
JB = S // 128            # 32 key blocks
CT = C // 128            # 2 channel tiles

F32 = mybir.dt.float32
BF16 = mybir.dt.bfloat16
I16 = mybir.dt.int16
MM_DT = BF16

# Schraudolph exp-as-bf16-bits: bits = round(s*SCH_S + SCH_B)
SCH_S = 128.0 * math.log2(math.e)
_r_max = 1.4426950408889634 / 2 ** 0.44269504088896340
SCH_B = 128.0 * 127.0 - 128.0 * (math.log2(_r_max) / 2.0)

USE_SCHRAUDOLPH = True   # h1 exp on DVE via int16 bit trick (else ACT exp)


def build_nc():
    nc = Bacc()
    x = nc.declare_dram_parameter("x", [C, S], F32, isOutput=False)
    xq = nc.declare_dram_parameter("xq", [C, SQ], F32, isOutput=False)
    proj_w = nc.declare_dram_parameter("proj_w", [3 * C, C], F32, isOutput=False)
    proj_b = nc.declare_dram_parameter("proj_b", [3 * C], F32, isOutput=False)
    out_w = nc.declare_dram_parameter("out_w", [C, C], F32, isOutput=False)
    out_b = nc.declare_dram_parameter("out_b", [C], F32, isOutput=False)
    gn_w = nc.declare_dram_parameter("gn_w", [C], F32, isOutput=False)
    gn_b = nc.declare_dram_parameter("gn_b", [C], F32, isOutput=False)
    out = nc.declare_dram_parameter("out", [C, SQ], F32, isOutput=True)

    with tile.TileContext(nc) as tc:
        _emit(nc, tc, x, xq, proj_w, proj_b, out_w, out_b, gn_w, gn_b, out)
    nc.finalize()
    return nc


def _emit(nc, tc, x, xq, proj_w, proj_b, out_w, out_b, gn_w, gn_b, out):
    with contextlib.ExitStack() as ctx:
        const = ctx.enter_context(tc.tile_pool(name="const", bufs=1))
        persist = ctx.enter_context(tc.tile_pool(name="persist", bufs=1))

        ident = const.tile([128, 128], F32)
        make_identity(nc, ident)

        # ---------------- phase 0: loads + stats (chunk-pipelined) ----------
        XCH = 4                  # x DMA chunks per tile
        CW = S // XCH            # 1024 columns per chunk
        xt = []
        for t in range(CT):
            xt_t = persist.tile([128, S], F32, name=f"xt{t}", tag=f"xt{t}")
            xt.append(xt_t)
        # interleave chunk DMAs across the two channel tiles
        for c in range(XCH):
            for t in range(CT):
                nc.sync.dma_start(
                    out=xt[t][:, c * CW:(c + 1) * CW],
                    in_=x[t * 128:(t + 1) * 128, c * CW:(c + 1) * CW])
        xqt = []
        for t in range(CT):
            xq_t = persist.tile([128, SQ], F32, name=f"xq{t}", tag=f"xq{t}")
            nc.sync.dma_start(out=xq_t, in_=xq[t * 128:(t + 1) * 128, :])
            xqt.append(xq_t)

        def load_col(dram_vec, lo, n, tag):
            col = const.tile([n, 1], F32, tag=tag)
            nc.gpsimd.dma_start(
                out=col, in_=dram_vec[lo:lo + n].rearrange("(p o) -> p o", o=1)
            )
            return col

        gnw = [load_col(gn_w, t * 128, 128, f"gnw{t}") for t in range(CT)]
        gnb = [load_col(gn_b, t * 128, 128, f"gnb{t}") for t in range(CT)]
        outb = [load_col(out_b, t * 128, 128, f"outb{t}") for t in range(CT)]
        # pair-stacked q/k biases [128, 1] (q pre-scaled by 1/8)
        qb2, kb2 = [], []
        for p in range(2):
            qbp = const.tile([128, 1], F32, name=f"qb2{p}", tag=f"qb2{p}")
            kbp = const.tile([128, 1], F32, name=f"kb2{p}", tag=f"kb2{p}")
            for hh in range(2):
                h = 2 * p + hh
                nc.sync.dma_start(
                    out=qbp[hh * 64:(hh + 1) * 64, :],
                    in_=proj_b[h * 192:h * 192 + 64]
                    .rearrange("(p o) -> p o", o=1))
                nc.sync.dma_start(
                    out=kbp[hh * 64:(hh + 1) * 64, :],
                    in_=proj_b[h * 192 + 64:h * 192 + 128]
                    .rearrange("(p o) -> p o", o=1))
            nc.vector.tensor_scalar_mul(out=qbp, in0=qbp, scalar1=SCALE2)
            qb2.append(qbp)
            kb2.append(kbp)
        # v bias pair-stacked [128, 1] for the out_w fold (bf16 for matmul)
        bv2 = []
        for p in range(2):
            bvp = const.tile([128, 1], F32, name=f"bv2{p}", tag=f"bv2{p}")
            for hh in range(2):
                h = 2 * p + hh
                nc.sync.dma_start(
                    out=bvp[hh * 64:(hh + 1) * 64, :],
                    in_=proj_b[h * 192 + 128:h * 192 + 192]
                    .rearrange("(p o) -> p o", o=1))
            bvb = const.tile([128, 1], MM_DT, name=f"bv2b{p}", tag=f"bv2b{p}")
            nc.vector.tensor_copy(out=bvb, in_=bvp)
            bv2.append(bvb)

        with tc.tile_pool(name="ph0", bufs=2) as ph0, \
             tc.tile_pool(name="ps0", bufs=4, space="PSUM") as ps0:

            # ---- weight loads + PE transposes (early PE work) ----
            # proj_w^T: pwT[ct][c_local, r] = proj_w[r, ct*128+c_local]
            pwT = [persist.tile([128, 3 * C], MM_DT, name=f"pwT{i}",
                                tag=f"pwT{i}") for i in range(CT)]
            for r in range(6):
                pw_r = ph0.tile([128, C], F32, name="pw", tag="pw")
                nc.sync.dma_start(out=pw_r, in_=proj_w[r * 128:(r + 1) * 128, :])
                for ct_i in range(CT):
                    pst = ps0.tile([128, 128], F32, name="tr", tag="ps0t")
                    nc.tensor.transpose(
                        out=pst, in_=pw_r[:, ct_i * 128:(ct_i + 1) * 128],
                        identity=ident)
                    nc.scalar.copy(out=pwT[ct_i][:, r * 128:(r + 1) * 128],
                                   in_=pst)
            # fold 1/8 into the q columns
            for ct_i in range(CT):
                qcols = pwT[ct_i].rearrange("p (h n) -> p h n", n=192)[:, :, 0:DK]
                nc.vector.tensor_scalar_mul(out=qcols, in0=qcols, scalar1=SCALE2)

            # v columns re-packed with a zero 65th column per head
            wvT = []
            for ct_i in range(CT):
                wv = persist.tile([128, NH * 65], MM_DT, name=f"wvT{ct_i}",
                                  tag=f"wvT{ct_i}")
                nc.gpsimd.memset(wv, 0.0)
                nc.vector.tensor_copy(
                    out=wv.rearrange("p (h n) -> p h n", n=65)[:, :, 0:DK],
                    in_=pwT[ct_i].rearrange("p (h n) -> p h n", n=192)
                    [:, :, 128:192],
                )
                wvT.append(wv)

            # out_w^T pair-packed: owT2[p][hh*64+d, c] = out_w[c, (2p+hh)*64+d]
            owT2 = [persist.tile([128, C], MM_DT, name=f"owT2{p}",
                                 tag=f"owT2{p}") for p in range(2)]
            for ct_i in range(CT):
                ow_c = ph0.tile([128, C], F32, name="ow", tag="ow")
                nc.sync.dma_start(out=ow_c,
                                  in_=out_w[ct_i * 128:(ct_i + 1) * 128, :])
                for h in range(NH):
                    p, hh = h // 2, h % 2
                    pst = ps0.tile([64, 128], F32, name="trh", tag="ps0t")
                    nc.tensor.transpose(
                        out=pst, in_=ow_c[:, h * 64:(h + 1) * 64],
                        identity=ident)
                    nc.scalar.copy(
                        out=owT2[p][hh * 64:(hh + 1) * 64,
                                    ct_i * 128:(ct_i + 1) * 128],
                        in_=pst)

            # ---- groupnorm stats (chunk-pipelined behind the x DMA) ----
            hb, hq = [], []
            scale_t, shift_t = [], []
            for t in range(CT):
                st6 = ph0.tile([128, 8, 6], F32, name=f"st6{t}", tag=f"st6{t}")
                xv = xt[t].rearrange("p (n f) -> p n f", f=512)
                for i in range(8):
                    nc.vector.bn_stats(out=st6[:, i, :], in_=xv[:, i, :])
                mv = ph0.tile([128, 2], F32, name="mv", tag=f"mv{t}")
                nc.vector.bn_aggr(out=mv, in_=st6)
                st2 = ph0.tile([128, 2], F32, name="st2", tag=f"st2{t}")
                sq = ph0.tile([128, 1], F32, name="sq", tag=f"sq{t}")
                nc.vector.tensor_mul(out=sq, in0=mv[:, 0:1], in1=mv[:, 0:1])
                nc.vector.tensor_copy(out=st2[:, 0:1], in_=mv[:, 0:1])
                nc.vector.tensor_add(out=st2[:, 1:2], in0=sq, in1=mv[:, 1:2])

                # group aggregation via one-hot(1/8) matmul -> [16, 2]
                gmat = ph0.tile([128, 16], F32, name="gmat", tag=f"gmat{t}")
                nc.gpsimd.memset(gmat, 0.125)
                nc.gpsimd.affine_select(
                    out=gmat, in_=gmat, compare_op=mybir.AluOpType.is_ge,
                    fill=0.0, base=0, pattern=[[-8, 16]], channel_multiplier=1)
                nc.gpsimd.affine_select(
                    out=gmat, in_=gmat, compare_op=mybir.AluOpType.is_ge,
                    fill=0.0, base=7, pattern=[[8, 16]], channel_multiplier=-1)
                ps_g = ps0.tile([16, 2], F32, name="psg", tag="ps0t")
                nc.tensor.matmul(out=ps_g, lhsT=gmat, rhs=st2,
                                 start=True, stop=True)
                gs = ph0.tile([16, 2], F32, name="gs", tag=f"gs{t}")
                nc.vector.tensor_copy(out=gs, in_=ps_g)

                # var_g = E[x^2]-mean^2; rstd = 1/sqrt(var+eps) + Newton polish
                sqg = ph0.tile([16, 1], F32, name="sqg", tag=f"sqg{t}")
                varg = ph0.tile([16, 1], F32, name="varg", tag=f"varg{t}")
                nc.vector.tensor_mul(out=sqg, in0=gs[:, 0:1], in1=gs[:, 0:1])
                nc.vector.tensor_sub(out=varg, in0=gs[:, 1:2], in1=sqg)
                epst = ph0.tile([16, 1], F32, name="epst", tag=f"epst{t}")
                nc.vector.memset(epst, EPS)
                srt = ph0.tile([16, 1], F32, name="srt", tag=f"srt{t}")
                nc.scalar.activation(out=srt, in_=varg,
                                     func=mybir.ActivationFunctionType.Sqrt,
                                     bias=epst, scale=1.0)
                r0 = ph0.tile([16, 1], F32, name="r0", tag=f"r0{t}")
                nc.vector.reciprocal(out=r0, in_=srt)
                ve = ph0.tile([16, 1], F32, name="ve", tag=f"ve{t}")
                nc.vector.tensor_scalar_add(out=ve, in0=varg, scalar1=EPS)
                r2 = ph0.tile([16, 1], F32, name="r2", tag=f"r2{t}")
                nc.vector.tensor_mul(out=r2, in0=r0, in1=r0)
                t1 = ph0.tile([16, 1], F32, name="t1", tag=f"t1{t}")
                nc.vector.tensor_mul(out=t1, in0=ve, in1=r2)
                t2 = ph0.tile([16, 1], F32, name="t2", tag=f"t2{t}")
                nc.vector.tensor_scalar(out=t2, in0=t1, scalar1=-0.5,
                                        scalar2=1.5,
                                        op0=mybir.AluOpType.mult,
                                        op1=mybir.AluOpType.add)
                rstd = ph0.tile([16, 1], F32, name="rstd", tag=f"rstd{t}")
                nc.vector.tensor_mul(out=rstd, in0=r0, in1=t2)

                # broadcast group params to channels with G^T one-hot matmul
                ps_gt = ps0.tile([16, 128], F32, name="psgt", tag="ps0t")
                nc.tensor.transpose(out=ps_gt, in_=gmat, identity=ident)
                g2 = ph0.tile([16, 128], F32, name="g2", tag=f"g2{t}")
                nc.scalar.mul(out=g2, in_=ps_gt, mul=8.0)
                grp2 = ph0.tile([16, 2], F32, name="grp2", tag=f"grp2{t}")
                nc.vector.tensor_copy(out=grp2[:, 0:1], in_=gs[:, 0:1])
                nc.vector.tensor_copy(out=grp2[:, 1:2], in_=rstd)
                ps_b = ps0.tile([128, 2], F32, name="psb", tag="ps0t")
                nc.tensor.matmul(out=ps_b, lhsT=g2, rhs=grp2,
                                 start=True, stop=True)
                chst = ph0.tile([128, 2], F32, name="chst", tag=f"chst{t}")
                nc.vector.tensor_copy(out=chst, in_=ps_b)

                # per-channel scale/shift with gamma/beta folded in
                scale = ph0.tile([128, 1], F32, name="scale", tag=f"scale{t}")
                nc.vector.tensor_mul(out=scale, in0=chst[:, 1:2], in1=gnw[t])
                tmp2 = ph0.tile([128, 1], F32, name="tmp2", tag=f"tmp2{t}")
                nc.vector.tensor_mul(out=tmp2, in0=chst[:, 0:1], in1=scale)
                shift = ph0.tile([128, 1], F32, name="shift", tag=f"shift{t}")
                nc.vector.tensor_sub(out=shift, in0=gnb[t], in1=tmp2)
                scale_t.append(scale)
                shift_t.append(shift)

                hb.append(persist.tile([128, S], MM_DT, name=f"hb{t}",
                                       tag=f"hb{t}"))
                hq.append(persist.tile([128, SQ], MM_DT, name=f"hq{t}",
                                       tag=f"hq{t}"))

            # normalized activations, emitted tile-interleaved per chunk so
            # the K projection can start on chunk 0 immediately
            for c in range(XCH):
                for t in range(CT):
                    sl = slice(c * CW, (c + 1) * CW)
                    nc.vector.tensor_scalar(out=hb[t][:, sl],
                                            in0=xt[t][:, sl],
                                            scalar1=scale_t[t],
                                            scalar2=shift_t[t],
                                            op0=mybir.AluOpType.mult,
                                            op1=mybir.AluOpType.add)
            for t in range(CT):
                nc.vector.tensor_scalar(out=hq[t], in0=xqt[t],
                                        scalar1=scale_t[t],
                                        scalar2=shift_t[t],
                                        op0=mybir.AluOpType.mult,
                                        op1=mybir.AluOpType.add)

        # ---------------- phase 1: projections ------------------------------
        kT2 = [persist.tile([128, S], MM_DT, name=f"kT2{p}", tag=f"kT2{p}")
               for p in range(2)]
        qT2 = [persist.tile([128, SQ], MM_DT, name=f"qT2{p}", tag=f"qT2{p}")
               for p in range(2)]
        vS = persist.tile([128, JB, NH * 65], MM_DT, name="vS")
        vS4 = vS.rearrange("p j (h n) -> p j h n", n=65)
        nc.gpsimd.memset(vS4[:, :, :, 64:65], 1.0)

        with tc.tile_pool(name="ps1k", bufs=3, space="PSUM") as ps1k, \
             tc.tile_pool(name="ps1v", bufs=3, space="PSUM") as ps1v:
            for p in range(2):
                for nb in range(S // 512):
                    ps_k = ps1k.tile([128, 512], F32, name="psk", tag="pskq")
                    for hh in range(2):
                        h = 2 * p + hh
                        wk_h = [pwT[i][:, h * 192 + 64:h * 192 + 128]
                                for i in range(CT)]
                        for i in range(CT):
                            nc.tensor.matmul(
                                out=ps_k[hh * 64:(hh + 1) * 64, :],
                                lhsT=wk_h[i],
                                rhs=hb[i][:, nb * 512:(nb + 1) * 512],
                                start=(i == 0), stop=(i == CT - 1),
                                tile_position=(0, hh * 64),
                                skip_group_check=True)
                    nc.scalar.add(out=kT2[p][:, nb * 512:(nb + 1) * 512],
                                  in_=ps_k, add=kb2[p])
                for nb in range(SQ // 512):
                    ps_q = ps1k.tile([128, 512], F32, name="psq", tag="pskq")
                    for hh in range(2):
                        h = 2 * p + hh
                        wq_h = [pwT[i][:, h * 192:h * 192 + 64]
                                for i in range(CT)]
                        for i in range(CT):
                            nc.tensor.matmul(
                                out=ps_q[hh * 64:(hh + 1) * 64, :],
                                lhsT=wq_h[i],
                                rhs=hq[i][:, nb * 512:(nb + 1) * 512],
                                start=(i == 0), stop=(i == CT - 1),
                                tile_position=(0, hh * 64),
                                skip_group_check=True)
                    nc.scalar.add(out=qT2[p][:, nb * 512:(nb + 1) * 512],
                                  in_=ps_q, add=qb2[p])

            # v in [S, dk] layout (65th column per head pre-set to ones)
            for jb in range(JB):
                ps_v = ps1v.tile([128, NH * 65], F32, name="psv", tag="psv")
                for i in range(CT):
                    nc.tensor.matmul(
                        out=ps_v, lhsT=hb[i][:, jb * 128:(jb + 1) * 128],
                        rhs=wvT[i], start=(i == 0), stop=(i == CT - 1))
                pv4 = ps_v.rearrange("p (h n) -> p h n", n=65)
                if jb % 2 == 0:
                    nc.scalar.copy(out=vS4[:, jb, :, 0:DK],
                                   in_=pv4[:, :, 0:DK])
                else:
                    nc.vector.tensor_copy(out=vS4[:, jb, :, 0:DK],
                                          in_=pv4[:, :, 0:DK])

        # ---------------- phase 2: attention --------------------------------
        resT2 = [persist.tile([128, SQ], MM_DT, name=f"res2{p}",
                              tag=f"res2{p}") for p in range(2)]
        rcp_dram = nc.dram_tensor("rcp_scratch", [2, 2 * SQ], F32)

        with tc.tile_pool(name="ps2s0", bufs=1, space="PSUM") as ps2s0, \
             tc.tile_pool(name="ps2s1", bufs=1, space="PSUM") as ps2s1, \
             tc.tile_pool(name="ps2o", bufs=1, space="PSUM") as ps2o, \
             tc.tile_pool(name="et", bufs=2) as etp, \
             tc.tile_pool(name="dn", bufs=1) as dnp:
            for p in range(2):
                po = ps2o.tile([65, 2, SQ], F32, name="po", tag="po")
                s_tiles = [None, None]
                e_tiles = [None, None]

                def scores(jb):
                    s0 = ps2s0.tile([128, SQ], F32, name="s0", tag="s0")
                    s1 = ps2s1.tile([128, SQ], F32, name="s1", tag="s1")
                    for ih in range(SQ // 512):
                        sl = slice(ih * 512, (ih + 1) * 512)
                        nc.tensor.matmul(
                            out=s0[:, sl],
                            lhsT=kT2[p][0:64, jb * 128:(jb + 1) * 128],
                            rhs=qT2[p][0:64, sl],
                            start=True, stop=True, skip_group_check=True)
                        nc.tensor.matmul(
                            out=s1[:, sl],
                            lhsT=kT2[p][64:128, jb * 128:(jb + 1) * 128],
                            rhs=qT2[p][64:128, sl],
                            start=True, stop=True, skip_group_check=True)
                    e0 = etp.tile([128, SQ], MM_DT, name="e0", tag="e0")
                    nc.scalar.activation(out=e0, in_=s0,
                                         func=mybir.ActivationFunctionType.Exp)
                    if USE_SCHRAUDOLPH:
                        e1 = etp.tile([128, SQ], MM_DT, name="e1", tag="e1")
                        nc.vector.tensor_scalar(out=e1.bitcast(I16), in0=s1,
                                                scalar1=SCH_S, scalar2=SCH_B,
                                                op0=mybir.AluOpType.mult,
                                                op1=mybir.AluOpType.add)
                    else:
                        e1 = etp.tile([128, SQ], MM_DT, name="e1", tag="e1")
                        nc.scalar.activation(
                            out=e1, in_=s1,
                            func=mybir.ActivationFunctionType.Exp)
                    e_tiles[0], e_tiles[1] = e0, e1

                def pv(jb, e0, e1):
                    for hh, e_t in ((0, e0), (1, e1)):
                        h = 2 * p + hh
                        for ih in range(SQ // 512):
                            sl = slice(ih * 512, (ih + 1) * 512)
                            nc.tensor.matmul(
                                out=po[:, hh, sl],
                                lhsT=vS[:, jb, h * 65:(h + 1) * 65],
                                rhs=e_t[:, sl],
                                start=(jb == 0), stop=(jb == JB - 1),
                                skip_group_check=True)

                prev = None
                for jb in range(JB):
                    scores(jb)
                    cur = (e_tiles[0], e_tiles[1])
                    if prev is not None:
                        pv(jb - 1, *prev)
                    prev = cur
                pv(JB - 1, *prev)

                # normalize: rcp of denominator row, DMA partition-broadcast
                rcp = dnp.tile([1, 2, SQ], F32, name="rcp", tag=f"rcp{p}")
                nc.vector.reciprocal(out=rcp, in_=po[64:65, :, :])
                nc.sync.dma_start(
                    out=rcp_dram[p, :].rearrange("(o n) -> o n", o=1),
                    in_=rcp.rearrange("o a n -> o (a n)"))
                rcpb = dnp.tile([64, 2, SQ], F32, name="rcpb", tag=f"rcpb{p}")
                # 4 queue-parallel broadcast DMAs, 16 dest partitions each
                rsrc = rcp_dram[p, :]
                for qd in range(4):
                    nc.sync.dma_start(
                        out=rcpb[qd * 16:(qd + 1) * 16, :, :],
                        in_=bass.AP(tensor=rsrc.tensor, offset=rsrc.offset,
                                    ap=[[0, 16], [1, 2 * SQ]]))
                for hh in range(2):
                    nc.vector.tensor_mul(
                        out=resT2[p][hh * 64:(hh + 1) * 64, :],
                        in0=po[0:64, hh, :], in1=rcpb[:, hh, :])

        # ---------------- phase 3: out-projection + residual ----------------
        with tc.tile_pool(name="ps3", bufs=1, space="PSUM") as ps3, \
             tc.tile_pool(name="ob", bufs=2) as obp:
            # fold the v-bias through out_w: wbv[c] = sum_hd out_w[c,hd]*bv[hd]
            ps_wbv = [ps3.tile([128, 1], F32, name=f"wbv{t2}", tag=f"wbv{t2}")
                      for t2 in range(CT)]
            for ct_i in range(CT):
                for p in range(2):
                    nc.tensor.matmul(
                        out=ps_wbv[ct_i],
                        lhsT=owT2[p][:, ct_i * 128:(ct_i + 1) * 128],
                        rhs=bv2[p], start=(p == 0), stop=(p == 1),
                        skip_group_check=True)
            outb2 = [obp.tile([128, 1], F32, name=f"ob2{t2}", tag=f"ob2{t2}")
                     for t2 in range(CT)]
            for ct_i in range(CT):
                nc.vector.tensor_add(out=outb2[ct_i], in0=outb[ct_i],
                                     in1=ps_wbv[ct_i])

            ps_outs = [ps3.tile([128, SQ], F32, name=f"pso3{t2}", tag="pso3")
                       for t2 in range(CT)]
            for p in range(2):
                for ct_i in range(CT):
                    for ih in range(SQ // 512):
                        sl = slice(ih * 512, (ih + 1) * 512)
                        nc.tensor.matmul(
                            out=ps_outs[ct_i][:, sl],
                            lhsT=owT2[p][:, ct_i * 128:(ct_i + 1) * 128],
                            rhs=resT2[p][:, sl],
                            start=(p == 0), stop=(p == 1),
                            skip_group_check=True)
            for ct_i in range(CT):
                obuf = obp.tile([128, SQ], F32, name="obuf", tag="obuf")
                nc.vector.scalar_tensor_tensor(
                    out=obuf, in0=ps_outs[ct_i], scalar=outb2[ct_i],
                    in1=xqt[ct_i],
                    op0=mybir.AluOpType.add, op1=mybir.AluOpType.add)
                nc.sync.dma_start(out=out[ct_i * 128:(ct_i + 1) * 128, :],
                                  in_=obuf)


_NC_CACHE = None


def _get_nc():
    global _NC_CACHE
    if _NC_CACHE is None:
        _NC_CACHE = build_nc()
    return _NC_CACHE


def _make_in_maps(x, gn_w, gn_b, proj_w, proj_b, out_w, out_b):
    xf = np.ascontiguousarray(np.asarray(x, dtype=np.float32)).reshape(B, C, S)
    shared = {
        "proj_w": np.ascontiguousarray(proj_w, dtype=np.float32),
        "proj_b": np.ascontiguousarray(proj_b, dtype=np.float32),
        "out_w": np.ascontiguousarray(out_w, dtype=np.float32),
        "out_b": np.ascontiguousarray(out_b, dtype=np.float32),
        "gn_w": np.ascontiguousarray(gn_w, dtype=np.float32),
        "gn_b": np.ascontiguousarray(gn_b, dtype=np.float32),
    }
    in_maps = []
    for core in range(N_CORES):
        b, chunk = core // CHUNKS, core % CHUNKS
        in_maps.append({
            "x": np.ascontiguousarray(xf[b]),
            "xq": np.ascontiguousarray(xf[b][:, chunk * SQ:(chunk + 1) * SQ]),
            **shared,
        })
    return in_maps


def _gather(results):
    outp = np.empty((B, C, S), dtype=np.float32)
    for core in range(N_CORES):
        b, chunk = core // CHUNKS, core % CHUNKS
        outp[b][:, chunk * SQ:(chunk + 1) * SQ] = results[core]["out"]
    return outp.reshape(B, C, H, W)


def kernel(x, gn_w, gn_b, proj_w, proj_b, out_w, out_b):
    import concourse.bass_utils as bu
    bu.upload_artifacts = lambda tmpdir: tmpdir  # no artifact bucket in sandbox

    in_maps = _make_in_maps(x, gn_w, gn_b, proj_w, proj_b, out_w, out_b)
    res = bu.run_bass_kernel_spmd(_get_nc(), in_maps, list(range(N_CORES)))
    return _gather(res.results)


# revision 17
# speedup vs baseline: 1.2302x; 1.2302x over previous
"""Trainium2 Bass kernel for nn_AttentionBlock (GroupNorm + 4-head self-attention
over S=4096 + output projection + residual) on x:[2, 256, 64, 64].

Sharding: 8 cores = (batch 2) x (query-chunk 4). Each core receives the full
image of its batch (for GroupNorm stats and K/V over all 4096 positions) plus
its own 1024-query chunk, and produces the exact [256, 1024] output slice.
No cross-core collectives.

v2 structure (per core):
  phase 0: chunked x DMA with bn_stats pipelined right behind each chunk;
           PE does weight transposes + the group-aggregation matmuls early.
           Per-channel scale/shift -> hb (full image) / hq (query chunk), bf16.
  phase 1: K/Q projections per head-pair (col-tiled, PSUM), V projection per
           128-key block; drains split across ScalarE / VectorE.
  phase 2: attention per head-pair. The two heads' score matmuls (K=dk=64)
           run CONCURRENTLY as 64x128 row-tiles (head A on PE rows 0:64,
           head B on rows 64:128 - the kT2/qT2 pair packing puts each head's
           operands on exactly those SBUF partitions). exp splits across
           engines: head A on ScalarE (native EXP), head B on VectorE via a
           Schraudolph bit-trick (y = int16(s*184.665+16250.5) bitcast bf16,
           +-3% per-weight; the softmax ratio cancels the systematic part).
           PV accumulates [65, 2, SQ] in one 4-bank PSUM tile (ones column
           -> denominators). PV(jb-1) is issued between S(jb) and S(jb+1) so
           the PE stays busy during the exp drain.
  phase 3: per pair: reciprocal of the denominator row, DMA partition-
           broadcast (SBUF->SBUF, no DRAM round trip), normalize into
           resT2[p] [128, SQ] (head pair stacked). Out-projection contracts
           both heads of a pair in single K=128 matmuls (owT2 pair-packed),
           accumulating both pairs into one PSUM group; + out_b (with the
           v-bias folded through out_w) + residual, DMA out.
"""

import contextlib
import math

import numpy as np

import concourse.bass as bass
import concourse.tile as tile
from concourse import mybir
from concourse.bacc import Bacc
from concourse.masks import make_identity

# Problem constants (hardcoded per harness contract).
B = 2
C = 256
H = W = 64
S = H * W            # 4096
NH = 4
DK = 64
EPS = 1e-5
SCALE2 = 0.125       # (1/sqrt(sqrt(dk)))^2, folded into Wq/bq
N_CORES = 8
CHUNKS = N_CORES // B    # query chunks per batch
SQ = S // CHUNKS         # queries per core (1024)
JB = S // 128            # 32 key blocks
CT = C // 128            # 2 channel tiles

F32 = mybir.dt.float32
BF16 = mybir.dt.bfloat16
I16 = mybir.dt.int16
MM_DT = BF16

# Schraudolph exp-as-bf16-bits: bits = round(s*SCH_S + SCH_B)
SCH_S = 128.0 * math.log2(math.e)
_r_max = 1.4426950408889634 / 2 ** 0.44269504088896340
SCH_B = 128.0 * 127.0 - 128.0 * (math.log2(_r_max) / 2.0)

USE_SCHRAUDOLPH = True   # h1 exp on DVE via int16 bit trick (else ACT exp)


def build_nc():
    nc = Bacc()
    x = nc.declare_dram_parameter("x", [C, S], F32, isOutput=False)
    xq = nc.declare_dram_parameter("xq", [C, SQ], F32, isOutput=False)
    proj_w = nc.declare_dram_parameter("proj_w", [3 * C, C], F32, isOutput=False)
    proj_b = nc.declare_dram_parameter("proj_b", [3 * C], F32, isOutput=False)
    out_w = nc.declare_dram_parameter("out_w", [C, C], F32, isOutput=False)
    out_b = nc.declare_dram_parameter("out_b", [C], F32, isOutput=False)
    gn_w = nc.declare_dram_parameter("gn_w", [C], F32, isOutput=False)
    gn_b = nc.declare_dram_parameter("gn_b", [C], F32, isOutput=False)
    out = nc.declare_dram_parameter("out", [C, SQ], F32, isOutput=True)

    with tile.TileContext(nc) as tc:
        _emit(nc, tc, x, xq, proj_w, proj_b, out_w, out_b, gn_w, gn_b, out)
    nc.finalize()
    return nc


def _emit(nc, tc, x, xq, proj_w, proj_b, out_w, out_b, gn_w, gn_b, out):
    with contextlib.ExitStack() as ctx:
        const = ctx.enter_context(tc.tile_pool(name="const", bufs=1))
        persist = ctx.enter_context(tc.tile_pool(name="persist", bufs=1))

        ident = const.tile([128, 128], F32)
        make_identity(nc, ident)

        # ---------------- phase 0: loads + stats (chunk-pipelined) ----------
        # weight DMAs go first (small; transposes are early PE work), then
        # the 5MB of activations
        XCH = 4                  # x DMA chunks per tile
        CW = S // XCH            # 1024 columns per chunk

        def load_col(dram_vec, lo, n, tag):
            col = const.tile([n, 1], F32, tag=tag)
            nc.gpsimd.dma_start(
                out=col, in_=dram_vec[lo:lo + n].rearrange("(p o) -> p o", o=1)
            )
            return col

        gnw = [load_col(gn_w, t * 128, 128, f"gnw{t}") for t in range(CT)]
        gnb = [load_col(gn_b, t * 128, 128, f"gnb{t}") for t in range(CT)]
        outb = [load_col(out_b, t * 128, 128, f"outb{t}") for t in range(CT)]
        # pair-stacked q/k biases [128, 1] (q pre-scaled by 1/8)
        qb2, kb2 = [], []
        for p in range(2):
            qbp = const.tile([128, 1], F32, name=f"qb2{p}", tag=f"qb2{p}")
            kbp = const.tile([128, 1], F32, name=f"kb2{p}", tag=f"kb2{p}")
            for hh in range(2):
                h = 2 * p + hh
                nc.sync.dma_start(
                    out=qbp[hh * 64:(hh + 1) * 64, :],
                    in_=proj_b[h * 192:h * 192 + 64]
                    .rearrange("(p o) -> p o", o=1))
                nc.sync.dma_start(
                    out=kbp[hh * 64:(hh + 1) * 64, :],
                    in_=proj_b[h * 192 + 64:h * 192 + 128]
                    .rearrange("(p o) -> p o", o=1))
            nc.vector.tensor_scalar_mul(out=qbp, in0=qbp, scalar1=SCALE2)
            qb2.append(qbp)
            kb2.append(kbp)
        # v bias pair-stacked [128, 1] for the out_w fold (bf16 for matmul)
        bv2 = []
        for p in range(2):
            bvp = const.tile([128, 1], F32, name=f"bv2{p}", tag=f"bv2{p}")
            for hh in range(2):
                h = 2 * p + hh
                nc.sync.dma_start(
                    out=bvp[hh * 64:(hh + 1) * 64, :],
                    in_=proj_b[h * 192 + 128:h * 192 + 192]
                    .rearrange("(p o) -> p o", o=1))
            bvb = const.tile([128, 1], MM_DT, name=f"bv2b{p}", tag=f"bv2b{p}")
            nc.vector.tensor_copy(out=bvb, in_=bvp)
            bv2.append(bvb)

        with tc.tile_pool(name="ph0", bufs=2) as ph0, \
             tc.tile_pool(name="pw0", bufs=3) as pw0, \
             tc.tile_pool(name="ps0", bufs=4, space="PSUM") as ps0:

            # ---- weight loads + PE transposes (early PE work) ----
            # proj_w^T: pwT[ct][c_local, r] = proj_w[r, ct*128+c_local]
            pwT = [persist.tile([128, 3 * C], MM_DT, name=f"pwT{i}",
                                tag=f"pwT{i}") for i in range(CT)]
            for r in range(6):
                pw_r = pw0.tile([128, C], F32, name="pw", tag="pw")
                nc.sync.dma_start(out=pw_r, in_=proj_w[r * 128:(r + 1) * 128, :])
                for ct_i in range(CT):
                    pst = ps0.tile([128, 128], F32, name="tr", tag="ps0t")
                    nc.tensor.transpose(
                        out=pst, in_=pw_r[:, ct_i * 128:(ct_i + 1) * 128],
                        identity=ident)
                    nc.scalar.copy(out=pwT[ct_i][:, r * 128:(r + 1) * 128],
                                   in_=pst)
            # fold 1/8 into the q columns
            for ct_i in range(CT):
                qcols = pwT[ct_i].rearrange("p (h n) -> p h n", n=192)[:, :, 0:DK]
                nc.vector.tensor_scalar_mul(out=qcols, in0=qcols, scalar1=SCALE2)

            # v columns re-packed with a zero 65th column per head
            wvT = []
            for ct_i in range(CT):
                wv = persist.tile([128, NH * 65], MM_DT, name=f"wvT{ct_i}",
                                  tag=f"wvT{ct_i}")
                nc.gpsimd.memset(wv, 0.0)
                nc.vector.tensor_copy(
                    out=wv.rearrange("p (h n) -> p h n", n=65)[:, :, 0:DK],
                    in_=pwT[ct_i].rearrange("p (h n) -> p h n", n=192)
                    [:, :, 128:192],
                )
                wvT.append(wv)

            # out_w^T pair-packed: owT2[p][hh*64+d, c] = out_w[c, (2p+hh)*64+d]
            owT2 = [persist.tile([128, C], MM_DT, name=f"owT2{p}",
                                 tag=f"owT2{p}") for p in range(2)]
            for ct_i in range(CT):
                ow_c = ph0.tile([128, C], F32, name="ow", tag="ow")
                nc.sync.dma_start(out=ow_c,
                                  in_=out_w[ct_i * 128:(ct_i + 1) * 128, :])
                for h in range(NH):
                    p, hh = h // 2, h % 2
                    pst = ps0.tile([64, 128], F32, name="trh", tag="ps0t")
                    nc.tensor.transpose(
                        out=pst, in_=ow_c[:, h * 64:(h + 1) * 64],
                        identity=ident)
                    nc.scalar.copy(
                        out=owT2[p][hh * 64:(hh + 1) * 64,
                                    ct_i * 128:(ct_i + 1) * 128],
                        in_=pst)

            # ---- activation DMAs (after the weights are queued) ----
            xt = []
            for t in range(CT):
                xt_t = persist.tile([128, S], F32, name=f"xt{t}", tag=f"xt{t}")
                xt.append(xt_t)
            for c in range(XCH):
                for t in range(CT):
                    nc.sync.dma_start(
                        out=xt[t][:, c * CW:(c + 1) * CW],
                        in_=x[t * 128:(t + 1) * 128, c * CW:(c + 1) * CW])
            xqt = []
            for t in range(CT):
                xq_t = persist.tile([128, SQ], F32, name=f"xq{t}",
                                    tag=f"xq{t}")
                nc.sync.dma_start(out=xq_t, in_=xq[t * 128:(t + 1) * 128, :])
                xqt.append(xq_t)

            # ---- groupnorm stats (chunk-pipelined behind the x DMA) ----
            hb, hq = [], []
            scale_t, shift_t = [], []
            for t in range(CT):
                st6 = ph0.tile([128, 8, 6], F32, name=f"st6{t}", tag=f"st6{t}")
                xv = xt[t].rearrange("p (n f) -> p n f", f=512)
                for i in range(8):
                    nc.vector.bn_stats(out=st6[:, i, :], in_=xv[:, i, :])
                mv = ph0.tile([128, 2], F32, name="mv", tag=f"mv{t}")
                nc.vector.bn_aggr(out=mv, in_=st6)
                st2 = ph0.tile([128, 2], F32, name="st2", tag=f"st2{t}")
                sq = ph0.tile([128, 1], F32, name="sq", tag=f"sq{t}")
                nc.vector.tensor_mul(out=sq, in0=mv[:, 0:1], in1=mv[:, 0:1])
                nc.vector.tensor_copy(out=st2[:, 0:1], in_=mv[:, 0:1])
                nc.vector.tensor_add(out=st2[:, 1:2], in0=sq, in1=mv[:, 1:2])

                # group aggregation via one-hot(1/8) matmul -> [16, 2]
                gmat = ph0.tile([128, 16], F32, name="gmat", tag=f"gmat{t}")
                nc.gpsimd.memset(gmat, 0.125)
                nc.gpsimd.affine_select(
                    out=gmat, in_=gmat, compare_op=mybir.AluOpType.is_ge,
                    fill=0.0, base=0, pattern=[[-8, 16]], channel_multiplier=1)
                nc.gpsimd.affine_select(
                    out=gmat, in_=gmat, compare_op=mybir.AluOpType.is_ge,
                    fill=0.0, base=7, pattern=[[8, 16]], channel_multiplier=-1)
                ps_g = ps0.tile([16, 2], F32, name="psg", tag="ps0t")
                nc.tensor.matmul(out=ps_g, lhsT=gmat, rhs=st2,
                                 start=True, stop=True)
                gs = ph0.tile([16, 2], F32, name="gs", tag=f"gs{t}")
                nc.vector.tensor_copy(out=gs, in_=ps_g)

                # var_g = E[x^2]-mean^2; rstd = 1/sqrt(var+eps) + Newton polish
                sqg = ph0.tile([16, 1], F32, name="sqg", tag=f"sqg{t}")
                varg = ph0.tile([16, 1], F32, name="varg", tag=f"varg{t}")
                nc.vector.tensor_mul(out=sqg, in0=gs[:, 0:1], in1=gs[:, 0:1])
                nc.vector.tensor_sub(out=varg, in0=gs[:, 1:2], in1=sqg)
                epst = ph0.tile([16, 1], F32, name="epst", tag=f"epst{t}")
                nc.vector.memset(epst, EPS)
                srt = ph0.tile([16, 1], F32, name="srt", tag=f"srt{t}")
                nc.scalar.activation(out=srt, in_=varg,
                                     func=mybir.ActivationFunctionType.Sqrt,
                                     bias=epst, scale=1.0)
                r0 = ph0.tile([16, 1], F32, name="r0", tag=f"r0{t}")
                nc.vector.reciprocal(out=r0, in_=srt)
                ve = ph0.tile([16, 1], F32, name="ve", tag=f"ve{t}")
                nc.vector.tensor_scalar_add(out=ve, in0=varg, scalar1=EPS)
                r2 = ph0.tile([16, 1], F32, name="r2", tag=f"r2{t}")
                nc.vector.tensor_mul(out=r2, in0=r0, in1=r0)
                t1 = ph0.tile([16, 1], F32, name="t1", tag=f"t1{t}")
                nc.vector.tensor_mul(out=t1, in0=ve, in1=r2)
                t2 = ph0.tile([16, 1], F32, name="t2", tag=f"t2{t}")
                nc.vector.tensor_scalar(out=t2, in0=t1, scalar1=-0.5,
                                        scalar2=1.5,
                                        op0=mybir.AluOpType.mult,
                                        op1=mybir.AluOpType.add)
                rstd = ph0.tile([16, 1], F32, name="rstd", tag=f"rstd{t}")
                nc.vector.tensor_mul(out=rstd, in0=r0, in1=t2)

                # broadcast group params to channels with G^T one-hot matmul
                ps_gt = ps0.tile([16, 128], F32, name="psgt", tag="ps0t")
                nc.tensor.transpose(out=ps_gt, in_=gmat, identity=ident)
                g2 = ph0.tile([16, 128], F32, name="g2", tag=f"g2{t}")
                nc.scalar.mul(out=g2, in_=ps_gt, mul=8.0)
                grp2 = ph0.tile([16, 2], F32, name="grp2", tag=f"grp2{t}")
                nc.vector.tensor_copy(out=grp2[:, 0:1], in_=gs[:, 0:1])
                nc.vector.tensor_copy(out=grp2[:, 1:2], in_=rstd)
                ps_b = ps0.tile([128, 2], F32, name="psb", tag="ps0t")
                nc.tensor.matmul(out=ps_b, lhsT=g2, rhs=grp2,
                                 start=True, stop=True)
                chst = ph0.tile([128, 2], F32, name="chst", tag=f"chst{t}")
                nc.vector.tensor_copy(out=chst, in_=ps_b)

                # per-channel scale/shift with gamma/beta folded in
                scale = ph0.tile([128, 1], F32, name="scale", tag=f"scale{t}")
                nc.vector.tensor_mul(out=scale, in0=chst[:, 1:2], in1=gnw[t])
                tmp2 = ph0.tile([128, 1], F32, name="tmp2", tag=f"tmp2{t}")
                nc.vector.tensor_mul(out=tmp2, in0=chst[:, 0:1], in1=scale)
                shift = ph0.tile([128, 1], F32, name="shift", tag=f"shift{t}")
                nc.vector.tensor_sub(out=shift, in0=gnb[t], in1=tmp2)
                scale_t.append(scale)
                shift_t.append(shift)

                hb.append(persist.tile([128, S], MM_DT, name=f"hb{t}",
                                       tag=f"hb{t}"))
                hq.append(persist.tile([128, SQ], MM_DT, name=f"hq{t}",
                                       tag=f"hq{t}"))

            # normalized activations; hq first (the Q projection is first
            # consumer), then hb tile-interleaved per chunk
            for t in range(CT):
                nc.vector.tensor_scalar(out=hq[t], in0=xqt[t],
                                        scalar1=scale_t[t],
                                        scalar2=shift_t[t],
                                        op0=mybir.AluOpType.mult,
                                        op1=mybir.AluOpType.add)
            for c in range(XCH):
                for t in range(CT):
                    sl = slice(c * CW, (c + 1) * CW)
                    nc.vector.tensor_scalar(out=hb[t][:, sl],
                                            in0=xt[t][:, sl],
                                            scalar1=scale_t[t],
                                            scalar2=shift_t[t],
                                            op0=mybir.AluOpType.mult,
                                            op1=mybir.AluOpType.add)

        # ---------------- phase 1: projections ------------------------------
        kT2 = [persist.tile([128, S], MM_DT, name=f"kT2{p}", tag=f"kT2{p}")
               for p in range(2)]
        qT2 = [persist.tile([128, SQ], MM_DT, name=f"qT2{p}", tag=f"qT2{p}")
               for p in range(2)]
        vS = persist.tile([128, JB, NH * 65], MM_DT, name="vS")
        vS4 = vS.rearrange("p j (h n) -> p j h n", n=65)
        nc.gpsimd.memset(vS4[:, :, :, 64:65], 1.0)

        with tc.tile_pool(name="ps1k", bufs=4, space="PSUM") as ps1k, \
             tc.tile_pool(name="ps1v", bufs=3, space="PSUM") as ps1v:

            def kq_group(p, nbs, wsel, src, dst, bias):
                # weight-resident sweep: psum tiles for all nbs held across
                # the (ct, hh) weight loads -> 16 dense back-to-back matmuls
                tiles = {nb: ps1k.tile([128, 512], F32, name="psk", tag="pskq")
                         for nb in nbs}
                for i in range(CT):
                    for hh in range(2):
                        h = 2 * p + hh
                        w = pwT[i][:, h * 192 + wsel:h * 192 + wsel + 64]
                        for nb in nbs:
                            nc.tensor.matmul(
                                out=tiles[nb][hh * 64:(hh + 1) * 64, :],
                                lhsT=w,
                                rhs=src[i][:, nb * 512:(nb + 1) * 512],
                                start=(i == 0), stop=(i == CT - 1),
                                tile_position=(0, hh * 64),
                                skip_group_check=True)
                for nb in nbs:
                    nc.scalar.add(out=dst[:, nb * 512:(nb + 1) * 512],
                                  in_=tiles[nb], add=bias)

            for p in range(2):
                kq_group(p, range(2), 0, hq, qT2[p], qb2[p])      # Q first
                kq_group(p, range(4), 64, hb, kT2[p], kb2[p])
                kq_group(p, range(4, 8), 64, hb, kT2[p], kb2[p])

            # v in [S, dk] layout (65th column per head pre-set to ones)
            for jb in range(JB):
                ps_v = ps1v.tile([128, NH * 65], F32, name="psv", tag="psv")
                for i in range(CT):
                    nc.tensor.matmul(
                        out=ps_v, lhsT=hb[i][:, jb * 128:(jb + 1) * 128],
                        rhs=wvT[i], start=(i == 0), stop=(i == CT - 1))
                pv4 = ps_v.rearrange("p (h n) -> p h n", n=65)
                if jb % 2 == 0:
                    nc.scalar.copy(out=vS4[:, jb, :, 0:DK],
                                   in_=pv4[:, :, 0:DK])
                else:
                    nc.vector.tensor_copy(out=vS4[:, jb, :, 0:DK],
                                          in_=pv4[:, :, 0:DK])

        # ---------------- phase 2: attention --------------------------------
        resT2 = [persist.tile([128, SQ], MM_DT, name=f"res2{p}",
                              tag=f"res2{p}") for p in range(2)]
        rcp_dram = nc.dram_tensor("rcp_scratch", [2, 2 * SQ], F32)

        with tc.tile_pool(name="ps2s0", bufs=1, space="PSUM") as ps2s0, \
             tc.tile_pool(name="ps2s1", bufs=1, space="PSUM") as ps2s1, \
             tc.tile_pool(name="ps2o", bufs=1, space="PSUM") as ps2o, \
             tc.tile_pool(name="et", bufs=3) as etp, \
             tc.tile_pool(name="dn", bufs=1) as dnp:
            for p in range(2):
                po = ps2o.tile([65, 2, SQ], F32, name="po", tag="po")
                s_tiles = [None, None]
                e_tiles = [None, None]

                def scores(jb):
                    s0 = ps2s0.tile([128, SQ], F32, name="s0", tag="s0")
                    s1 = ps2s1.tile([128, SQ], F32, name="s1", tag="s1")
                    for ih in range(SQ // 512):
                        sl = slice(ih * 512, (ih + 1) * 512)
                        nc.tensor.matmul(
                            out=s0[:, sl],
                            lhsT=kT2[p][0:64, jb * 128:(jb + 1) * 128],
                            rhs=qT2[p][0:64, sl],
                            start=True, stop=True, skip_group_check=True)
                        nc.tensor.matmul(
                            out=s1[:, sl],
                            lhsT=kT2[p][64:128, jb * 128:(jb + 1) * 128],
                            rhs=qT2[p][64:128, sl],
                            start=True, stop=True, skip_group_check=True)
                    e0 = etp.tile([128, SQ], MM_DT, name="e0", tag="e0")
                    nc.scalar.activation(out=e0, in_=s0,
                                         func=mybir.ActivationFunctionType.Exp)
                    if USE_SCHRAUDOLPH:
                        e1 = etp.tile([128, SQ], MM_DT, name="e1", tag="e1")
                        nc.vector.tensor_scalar(out=e1.bitcast(I16), in0=s1,
                                                scalar1=SCH_S, scalar2=SCH_B,
                                                op0=mybir.AluOpType.mult,
                                                op1=mybir.AluOpType.add)
                    else:
                        e1 = etp.tile([128, SQ], MM_DT, name="e1", tag="e1")
                        nc.scalar.activation(
                            out=e1, in_=s1,
                            func=mybir.ActivationFunctionType.Exp)
                    e_tiles[0], e_tiles[1] = e0, e1

                def pv(jb, e0, e1):
                    for hh, e_t in ((0, e0), (1, e1)):
                        h = 2 * p + hh
                        for ih in range(SQ // 512):
                            sl = slice(ih * 512, (ih + 1) * 512)
                            nc.tensor.matmul(
                                out=po[:, hh, sl],
                                lhsT=vS[:, jb, h * 65:(h + 1) * 65],
                                rhs=e_t[:, sl],
                                start=(jb == 0), stop=(jb == JB - 1),
                                skip_group_check=True)

                prev = None
                for jb in range(JB):
                    scores(jb)
                    cur = (e_tiles[0], e_tiles[1])
                    if prev is not None:
                        pv(jb - 1, *prev)
                    prev = cur
                pv(JB - 1, *prev)

                # normalize: 1/den = exp(-ln den) on ScalarE (exp and ln share
                # one ACT table set; avoids the slow DVE reciprocal), then a
                # DMA partition-broadcast
                lnd = dnp.tile([1, 2, SQ], F32, name="lnd", tag=f"lnd{p}")
                nc.scalar.activation(out=lnd, in_=po[64:65, :, :],
                                     func=mybir.ActivationFunctionType.Ln)
                rcp = dnp.tile([1, 2, SQ], F32, name="rcp", tag=f"rcp{p}")
                nc.scalar.activation(out=rcp, in_=lnd,
                                     func=mybir.ActivationFunctionType.Exp,
                                     scale=-1.0)
                nc.sync.dma_start(
                    out=rcp_dram[p, :].rearrange("(o n) -> o n", o=1),
                    in_=rcp.rearrange("o a n -> o (a n)"))
                rcpb = dnp.tile([64, 2, SQ], F32, name="rcpb", tag=f"rcpb{p}")
                # 4 queue-parallel broadcast DMAs, 16 dest partitions each
                rsrc = rcp_dram[p, :]
                for qd in range(4):
                    nc.sync.dma_start(
                        out=rcpb[qd * 16:(qd + 1) * 16, :, :],
                        in_=bass.AP(tensor=rsrc.tensor, offset=rsrc.offset,
                                    ap=[[0, 16], [1, 2 * SQ]]))
                for hh in range(2):
                    nc.vector.tensor_mul(
                        out=resT2[p][hh * 64:(hh + 1) * 64, :],
                        in0=po[0:64, hh, :], in1=rcpb[:, hh, :])

        # ---------------- phase 3: out-projection + residual ----------------
        with tc.tile_pool(name="ps3", bufs=1, space="PSUM") as ps3, \
             tc.tile_pool(name="ob", bufs=2) as obp:
            # fold the v-bias through out_w: wbv[c] = sum_hd out_w[c,hd]*bv[hd]
            ps_wbv = [ps3.tile([128, 1], F32, name=f"wbv{t2}", tag=f"wbv{t2}")
                      for t2 in range(CT)]
            for ct_i in range(CT):
                for p in range(2):
                    nc.tensor.matmul(
                        out=ps_wbv[ct_i],
                        lhsT=owT2[p][:, ct_i * 128:(ct_i + 1) * 128],
                        rhs=bv2[p], start=(p == 0), stop=(p == 1),
                        skip_group_check=True)
            outb2 = [obp.tile([128, 1], F32, name=f"ob2{t2}", tag=f"ob2{t2}")
                     for t2 in range(CT)]
            for ct_i in range(CT):
                nc.vector.tensor_add(out=outb2[ct_i], in0=outb[ct_i],
                                     in1=ps_wbv[ct_i])

            ps_outs = [ps3.tile([128, SQ], F32, name=f"pso3{t2}", tag="pso3")
                       for t2 in range(CT)]
            for p in range(2):
                for ct_i in range(CT):
                    for ih in range(SQ // 512):
                        sl = slice(ih * 512, (ih + 1) * 512)
                        nc.tensor.matmul(
                            out=ps_outs[ct_i][:, sl],
                            lhsT=owT2[p][:, ct_i * 128:(ct_i + 1) * 128],
                            rhs=resT2[p][:, sl],
                            start=(p == 0), stop=(p == 1),
                            skip_group_check=True)
            for ct_i in range(CT):
                obuf = obp.tile([128, SQ], F32, name="obuf", tag="obuf")
                nc.vector.scalar_tensor_tensor(
                    out=obuf, in0=ps_outs[ct_i], scalar=outb2[ct_i],
                    in1=xqt[ct_i],
                    op0=mybir.AluOpType.add, op1=mybir.AluOpType.add)
                nc.sync.dma_start(out=out[ct_i * 128:(ct_i + 1) * 128, :],
                                  in_=obuf)


_NC_CACHE = None


def _get_nc():
    global _NC_CACHE
    if _NC_CACHE is None:
        _NC_CACHE = build_nc()
    return _NC_CACHE


def _make_in_maps(x, gn_w, gn_b, proj_w, proj_b, out_w, out_b):
    xf = np.ascontiguousarray(np.asarray(x, dtype=np.float32)).reshape(B, C, S)
    shared = {
        "proj_w": np.ascontiguousarray(proj_w, dtype=np.float32),
        "proj_b": np.ascontiguousarray(proj_b, dtype=np.float32),
        "out_w": np.ascontiguousarray(out_w, dtype=np.float32),
        "out_b": np.ascontiguousarray(out_b, dtype=np.float32),
        "gn_w": np.ascontiguousarray(gn_w, dtype=np.float32),
        "gn_b": np.ascontiguousarray(gn_b, dtype=np.float32),
    }
    in_maps = []
    for core in range(N_CORES):
        b, chunk = core // CHUNKS, core % CHUNKS
        in_maps.append({
            "x": np.ascontiguousarray(xf[b]),
            "xq": np.ascontiguousarray(xf[b][:, chunk * SQ:(chunk + 1) * SQ]),
            **shared,
        })
    return in_maps


def _gather(results):
    outp = np.empty((B, C, S), dtype=np.float32)
    for core in range(N_CORES):
        b, chunk = core // CHUNKS, core % CHUNKS
        outp[b][:, chunk * SQ:(chunk + 1) * SQ] = results[core]["out"]
    return outp.reshape(B, C, H, W)


def kernel(x, gn_w, gn_b, proj_w, proj_b, out_w, out_b):
    import concourse.bass_utils as bu
    bu.upload_artifacts = lambda tmpdir: tmpdir  # no artifact bucket in sandbox

    in_maps = _make_in_maps(x, gn_w, gn_b, proj_w, proj_b, out_w, out_b)
    res = bu.run_bass_kernel_spmd(_get_nc(), in_maps, list(range(N_CORES)))
    return _gather(res.results)


# revision 21
# speedup vs baseline: 1.2457x; 1.0126x over previous
"""Trainium2 Bass kernel for nn_AttentionBlock (GroupNorm + 4-head self-attention
over S=4096 + output projection + residual) on x:[2, 256, 64, 64].

Sharding: 8 cores = (batch 2) x (query-chunk 4). Each core receives the full
image of its batch (for GroupNorm stats and K/V over all 4096 positions) plus
its own 1024-query chunk, and produces the exact [256, 1024] output slice.
No cross-core collectives.

v2 structure (per core):
  phase 0: chunked x DMA with bn_stats pipelined right behind each chunk;
           PE does weight transposes + the group-aggregation matmuls early.
           Per-channel scale/shift -> hb (full image) / hq (query chunk), bf16.
  phase 1: K/Q projections per head-pair (col-tiled, PSUM), V projection per
           128-key block; drains split across ScalarE / VectorE.
  phase 2: attention per head-pair. The two heads' score matmuls (K=dk=64)
           run CONCURRENTLY as 64x128 row-tiles (head A on PE rows 0:64,
           head B on rows 64:128 - the kT2/qT2 pair packing puts each head's
           operands on exactly those SBUF partitions). exp splits across
           engines: head A on ScalarE (native EXP), head B on VectorE via a
           Schraudolph bit-trick (y = int16(s*184.665+16250.5) bitcast bf16,
           +-3% per-weight; the softmax ratio cancels the systematic part).
           PV accumulates [65, 2, SQ] in one 4-bank PSUM tile (ones column
           -> denominators). PV(jb-1) is issued between S(jb) and S(jb+1) so
           the PE stays busy during the exp drain.
  phase 3: per pair: reciprocal of the denominator row, DMA partition-
           broadcast (SBUF->SBUF, no DRAM round trip), normalize into
           resT2[p] [128, SQ] (head pair stacked). Out-projection contracts
           both heads of a pair in single K=128 matmuls (owT2 pair-packed),
           accumulating both pairs into one PSUM group; + out_b (with the
           v-bias folded through out_w) + residual, DMA out.
"""

import contextlib
import math

import numpy as np

import concourse.bass as bass
import concourse.tile as tile
from concourse import mybir
from concourse.bacc import Bacc
from concourse.masks import make_identity

# Problem constants (hardcoded per harness contract).
B = 2
C = 256
H = W = 64
S = H * W            # 4096
NH = 4
DK = 64
EPS = 1e-5
SCALE2 = 0.125       # (1/sqrt(sqrt(dk)))^2, folded into Wq/bq
N_CORES = 8
CHUNKS = N_CORES // B    # query chunks per batch
SQ = S // CHUNKS         # queries per core (1024)
JB = S // 128            # 32 key blocks
CT = C // 128            # 2 channel tiles

F32 = mybir.dt.float32
BF16 = mybir.dt.bfloat16
I16 = mybir.dt.int16
MM_DT = BF16

# Schraudolph exp-as-bf16-bits: bits = round(s*SCH_S + SCH_B)
SCH_S = 128.0 * math.log2(math.e)
_r_max = 1.4426950408889634 / 2 ** 0.44269504088896340
SCH_B = 128.0 * 127.0 - 128.0 * (math.log2(_r_max) / 2.0)

USE_SCHRAUDOLPH = True   # h1 exp on DVE via int16 bit trick (else ACT exp)


def build_nc():
    nc = Bacc()
    x = nc.declare_dram_parameter("x", [C, S], F32, isOutput=False)
    xq = nc.declare_dram_parameter("xq", [C, SQ], F32, isOutput=False)
    proj_w = nc.declare_dram_parameter("proj_w", [3 * C, C], F32, isOutput=False)
    proj_b = nc.declare_dram_parameter("proj_b", [3 * C], F32, isOutput=False)
    out_w = nc.declare_dram_parameter("out_w", [C, C], F32, isOutput=False)
    out_b = nc.declare_dram_parameter("out_b", [C], F32, isOutput=False)
    gn_w = nc.declare_dram_parameter("gn_w", [C], F32, isOutput=False)
    gn_b = nc.declare_dram_parameter("gn_b", [C], F32, isOutput=False)
    out = nc.declare_dram_parameter("out", [C, SQ], F32, isOutput=True)

    with tile.TileContext(nc) as tc:
        _emit(nc, tc, x, xq, proj_w, proj_b, out_w, out_b, gn_w, gn_b, out)
    nc.finalize()
    return nc


def _emit(nc, tc, x, xq, proj_w, proj_b, out_w, out_b, gn_w, gn_b, out):
    with contextlib.ExitStack() as ctx:
        const = ctx.enter_context(tc.tile_pool(name="const", bufs=1))
        persist = ctx.enter_context(tc.tile_pool(name="persist", bufs=1))

        ident = const.tile([128, 128], F32)
        make_identity(nc, ident)

        # ---------------- phase 0: loads + stats (chunk-pipelined) ----------
        # weight DMAs go first (small; transposes are early PE work), then
        # the 5MB of activations
        XCH = 4                  # x DMA chunks per tile
        CW = S // XCH            # 1024 columns per chunk

        def load_col(dram_vec, lo, n, tag):
            col = const.tile([n, 1], F32, tag=tag)
            nc.gpsimd.dma_start(
                out=col, in_=dram_vec[lo:lo + n].rearrange("(p o) -> p o", o=1)
            )
            return col

        gnw = [load_col(gn_w, t * 128, 128, f"gnw{t}") for t in range(CT)]
        gnb = [load_col(gn_b, t * 128, 128, f"gnb{t}") for t in range(CT)]
        outb = [load_col(out_b, t * 128, 128, f"outb{t}") for t in range(CT)]
        # pair-stacked q/k biases [128, 1] (q pre-scaled by 1/8)
        qb2, kb2 = [], []
        for p in range(2):
            qbp = const.tile([128, 1], F32, name=f"qb2{p}", tag=f"qb2{p}")
            kbp = const.tile([128, 1], F32, name=f"kb2{p}", tag=f"kb2{p}")
            for hh in range(2):
                h = 2 * p + hh
                nc.sync.dma_start(
                    out=qbp[hh * 64:(hh + 1) * 64, :],
                    in_=proj_b[h * 192:h * 192 + 64]
                    .rearrange("(p o) -> p o", o=1))
                nc.sync.dma_start(
                    out=kbp[hh * 64:(hh + 1) * 64, :],
                    in_=proj_b[h * 192 + 64:h * 192 + 128]
                    .rearrange("(p o) -> p o", o=1))
            nc.vector.tensor_scalar_mul(out=qbp, in0=qbp, scalar1=SCALE2)
            qb2.append(qbp)
            kb2.append(kbp)
        # v bias pair-stacked [128, 1] for the out_w fold (bf16 for matmul)
        bv2 = []
        for p in range(2):
            bvp = const.tile([128, 1], F32, name=f"bv2{p}", tag=f"bv2{p}")
            for hh in range(2):
                h = 2 * p + hh
                nc.sync.dma_start(
                    out=bvp[hh * 64:(hh + 1) * 64, :],
                    in_=proj_b[h * 192 + 128:h * 192 + 192]
                    .rearrange("(p o) -> p o", o=1))
            bvb = const.tile([128, 1], MM_DT, name=f"bv2b{p}", tag=f"bv2b{p}")
            nc.vector.tensor_copy(out=bvb, in_=bvp)
            bv2.append(bvb)

        with tc.tile_pool(name="ph0", bufs=2) as ph0, \
             tc.tile_pool(name="pw0", bufs=3) as pw0, \
             tc.tile_pool(name="ps0", bufs=4, space="PSUM") as ps0:

            # ---- weight loads + PE transposes (early PE work) ----
            # proj_w^T: pwT[ct][c_local, r] = proj_w[r, ct*128+c_local]
            pwT = [persist.tile([128, 3 * C], MM_DT, name=f"pwT{i}",
                                tag=f"pwT{i}") for i in range(CT)]
            for r in range(6):
                pw_r = pw0.tile([128, C], F32, name="pw", tag="pw")
                nc.sync.dma_start(out=pw_r, in_=proj_w[r * 128:(r + 1) * 128, :])
                for ct_i in range(CT):
                    pst = ps0.tile([128, 128], F32, name="tr", tag="ps0t")
                    nc.tensor.transpose(
                        out=pst, in_=pw_r[:, ct_i * 128:(ct_i + 1) * 128],
                        identity=ident)
                    nc.scalar.copy(out=pwT[ct_i][:, r * 128:(r + 1) * 128],
                                   in_=pst)
            # fold 1/8 into the q columns
            for ct_i in range(CT):
                qcols = pwT[ct_i].rearrange("p (h n) -> p h n", n=192)[:, :, 0:DK]
                nc.vector.tensor_scalar_mul(out=qcols, in0=qcols, scalar1=SCALE2)

            # v columns re-packed with a zero 65th column per head
            wvT = []
            for ct_i in range(CT):
                wv = persist.tile([128, NH * 65], MM_DT, name=f"wvT{ct_i}",
                                  tag=f"wvT{ct_i}")
                nc.gpsimd.memset(wv, 0.0)
                nc.vector.tensor_copy(
                    out=wv.rearrange("p (h n) -> p h n", n=65)[:, :, 0:DK],
                    in_=pwT[ct_i].rearrange("p (h n) -> p h n", n=192)
                    [:, :, 128:192],
                )
                wvT.append(wv)

            # out_w^T pair-packed: owT2[p][hh*64+d, c] = out_w[c, (2p+hh)*64+d]
            owT2 = [persist.tile([128, C], MM_DT, name=f"owT2{p}",
                                 tag=f"owT2{p}") for p in range(2)]
            for ct_i in range(CT):
                ow_c = ph0.tile([128, C], F32, name="ow", tag="ow")
                nc.sync.dma_start(out=ow_c,
                                  in_=out_w[ct_i * 128:(ct_i + 1) * 128, :])
                for h in range(NH):
                    p, hh = h // 2, h % 2
                    pst = ps0.tile([64, 128], F32, name="trh", tag="ps0t")
                    nc.tensor.transpose(
                        out=pst, in_=ow_c[:, h * 64:(h + 1) * 64],
                        identity=ident)
                    nc.scalar.copy(
                        out=owT2[p][hh * 64:(hh + 1) * 64,
                                    ct_i * 128:(ct_i + 1) * 128],
                        in_=pst)

            # ---- activation DMAs (after the weights are queued) ----
            xt = []
            for t in range(CT):
                xt_t = persist.tile([128, S], F32, name=f"xt{t}", tag=f"xt{t}")
                xt.append(xt_t)
            for c in range(XCH):
                for t in range(CT):
                    nc.sync.dma_start(
                        out=xt[t][:, c * CW:(c + 1) * CW],
                        in_=x[t * 128:(t + 1) * 128, c * CW:(c + 1) * CW])
            xqt = []
            for t in range(CT):
                xq_t = persist.tile([128, SQ], F32, name=f"xq{t}",
                                    tag=f"xq{t}")
                nc.sync.dma_start(out=xq_t, in_=xq[t * 128:(t + 1) * 128, :])
                xqt.append(xq_t)

            # ---- groupnorm stats (chunk-pipelined behind the x DMA) ----
            hb, hq = [], []
            scale_t, shift_t = [], []
            for t in range(CT):
                st6 = ph0.tile([128, 8, 6], F32, name=f"st6{t}", tag=f"st6{t}")
                xv = xt[t].rearrange("p (n f) -> p n f", f=512)
                for i in range(8):
                    nc.vector.bn_stats(out=st6[:, i, :], in_=xv[:, i, :])
                mv = ph0.tile([128, 2], F32, name="mv", tag=f"mv{t}")
                nc.vector.bn_aggr(out=mv, in_=st6)
                st2 = ph0.tile([128, 2], F32, name="st2", tag=f"st2{t}")
                sq = ph0.tile([128, 1], F32, name="sq", tag=f"sq{t}")
                nc.vector.tensor_mul(out=sq, in0=mv[:, 0:1], in1=mv[:, 0:1])
                nc.vector.tensor_copy(out=st2[:, 0:1], in_=mv[:, 0:1])
                nc.vector.tensor_add(out=st2[:, 1:2], in0=sq, in1=mv[:, 1:2])

                # group aggregation via one-hot(1/8) matmul -> [16, 2]
                gmat = ph0.tile([128, 16], F32, name="gmat", tag=f"gmat{t}")
                nc.gpsimd.memset(gmat, 0.125)
                nc.gpsimd.affine_select(
                    out=gmat, in_=gmat, compare_op=mybir.AluOpType.is_ge,
                    fill=0.0, base=0, pattern=[[-8, 16]], channel_multiplier=1)
                nc.gpsimd.affine_select(
                    out=gmat, in_=gmat, compare_op=mybir.AluOpType.is_ge,
                    fill=0.0, base=7, pattern=[[8, 16]], channel_multiplier=-1)
                ps_g = ps0.tile([16, 2], F32, name="psg", tag="ps0t")
                nc.tensor.matmul(out=ps_g, lhsT=gmat, rhs=st2,
                                 start=True, stop=True)
                gs = ph0.tile([16, 2], F32, name="gs", tag=f"gs{t}")
                nc.vector.tensor_copy(out=gs, in_=ps_g)

                # var_g = E[x^2]-mean^2; rstd = 1/sqrt(var+eps) + Newton polish
                sqg = ph0.tile([16, 1], F32, name="sqg", tag=f"sqg{t}")
                varg = ph0.tile([16, 1], F32, name="varg", tag=f"varg{t}")
                nc.vector.tensor_mul(out=sqg, in0=gs[:, 0:1], in1=gs[:, 0:1])
                nc.vector.tensor_sub(out=varg, in0=gs[:, 1:2], in1=sqg)
                epst = ph0.tile([16, 1], F32, name="epst", tag=f"epst{t}")
                nc.vector.memset(epst, EPS)
                srt = ph0.tile([16, 1], F32, name="srt", tag=f"srt{t}")
                nc.scalar.activation(out=srt, in_=varg,
                                     func=mybir.ActivationFunctionType.Sqrt,
                                     bias=epst, scale=1.0)
                r0 = ph0.tile([16, 1], F32, name="r0", tag=f"r0{t}")
                nc.vector.reciprocal(out=r0, in_=srt)
                ve = ph0.tile([16, 1], F32, name="ve", tag=f"ve{t}")
                nc.vector.tensor_scalar_add(out=ve, in0=varg, scalar1=EPS)
                r2 = ph0.tile([16, 1], F32, name="r2", tag=f"r2{t}")
                nc.vector.tensor_mul(out=r2, in0=r0, in1=r0)
                t1 = ph0.tile([16, 1], F32, name="t1", tag=f"t1{t}")
                nc.vector.tensor_mul(out=t1, in0=ve, in1=r2)
                t2 = ph0.tile([16, 1], F32, name="t2", tag=f"t2{t}")
                nc.vector.tensor_scalar(out=t2, in0=t1, scalar1=-0.5,
                                        scalar2=1.5,
                                        op0=mybir.AluOpType.mult,
                                        op1=mybir.AluOpType.add)
                rstd = ph0.tile([16, 1], F32, name="rstd", tag=f"rstd{t}")
                nc.vector.tensor_mul(out=rstd, in0=r0, in1=t2)

                # broadcast group params to channels with G^T one-hot matmul
                ps_gt = ps0.tile([16, 128], F32, name="psgt", tag="ps0t")
                nc.tensor.transpose(out=ps_gt, in_=gmat, identity=ident)
                g2 = ph0.tile([16, 128], F32, name="g2", tag=f"g2{t}")
                nc.scalar.mul(out=g2, in_=ps_gt, mul=8.0)
                grp2 = ph0.tile([16, 2], F32, name="grp2", tag=f"grp2{t}")
                nc.vector.tensor_copy(out=grp2[:, 0:1], in_=gs[:, 0:1])
                nc.vector.tensor_copy(out=grp2[:, 1:2], in_=rstd)
                ps_b = ps0.tile([128, 2], F32, name="psb", tag="ps0t")
                nc.tensor.matmul(out=ps_b, lhsT=g2, rhs=grp2,
                                 start=True, stop=True)
                chst = ph0.tile([128, 2], F32, name="chst", tag=f"chst{t}")
                nc.vector.tensor_copy(out=chst, in_=ps_b)

                # per-channel scale/shift with gamma/beta folded in
                scale = ph0.tile([128, 1], F32, name="scale", tag=f"scale{t}")
                nc.vector.tensor_mul(out=scale, in0=chst[:, 1:2], in1=gnw[t])
                tmp2 = ph0.tile([128, 1], F32, name="tmp2", tag=f"tmp2{t}")
                nc.vector.tensor_mul(out=tmp2, in0=chst[:, 0:1], in1=scale)
                shift = ph0.tile([128, 1], F32, name="shift", tag=f"shift{t}")
                nc.vector.tensor_sub(out=shift, in0=gnb[t], in1=tmp2)
                scale_t.append(scale)
                shift_t.append(shift)

                hb.append(persist.tile([128, S], MM_DT, name=f"hb{t}",
                                       tag=f"hb{t}"))
                hq.append(persist.tile([128, SQ], MM_DT, name=f"hq{t}",
                                       tag=f"hq{t}"))

            # normalized activations; hq first (the Q projection is first
            # consumer), then hb tile-interleaved per chunk. Tile 0 on
            # VectorE, tile 1 on ScalarE (Identity with per-channel
            # scale/bias) so the chunks land ~2x faster.
            def norm_chunk(t, dst, src):
                if t == 0:
                    nc.vector.tensor_scalar(out=dst, in0=src,
                                            scalar1=scale_t[t],
                                            scalar2=shift_t[t],
                                            op0=mybir.AluOpType.mult,
                                            op1=mybir.AluOpType.add)
                else:
                    nc.scalar.activation(
                        out=dst, in_=src,
                        func=mybir.ActivationFunctionType.Identity,
                        bias=shift_t[t], scale=scale_t[t])

            for t in range(CT):
                norm_chunk(t, hq[t], xqt[t])
            for c in range(XCH):
                for t in range(CT):
                    sl = slice(c * CW, (c + 1) * CW)
                    norm_chunk(t, hb[t][:, sl], xt[t][:, sl])

        # ---------------- phase 1: projections ------------------------------
        kT2 = [persist.tile([128, S], MM_DT, name=f"kT2{p}", tag=f"kT2{p}")
               for p in range(2)]
        qT2 = [persist.tile([128, SQ], MM_DT, name=f"qT2{p}", tag=f"qT2{p}")
               for p in range(2)]
        vS = persist.tile([128, JB, NH * 65], MM_DT, name="vS")
        vS4 = vS.rearrange("p j (h n) -> p j h n", n=65)
        nc.gpsimd.memset(vS4[:, :, :, 64:65], 1.0)

        with tc.tile_pool(name="ps1k", bufs=4, space="PSUM") as ps1k, \
             tc.tile_pool(name="ps1v", bufs=3, space="PSUM") as ps1v:

            def kq_group(p, nbs, wsel, src, dst, bias):
                # weight-resident sweep: psum tiles for all nbs held across
                # the (ct, hh) weight loads -> 16 dense back-to-back matmuls
                tiles = {nb: ps1k.tile([128, 512], F32, name="psk", tag="pskq")
                         for nb in nbs}
                for i in range(CT):
                    for hh in range(2):
                        h = 2 * p + hh
                        w = pwT[i][:, h * 192 + wsel:h * 192 + wsel + 64]
                        for nb in nbs:
                            nc.tensor.matmul(
                                out=tiles[nb][hh * 64:(hh + 1) * 64, :],
                                lhsT=w,
                                rhs=src[i][:, nb * 512:(nb + 1) * 512],
                                start=(i == 0), stop=(i == CT - 1),
                                tile_position=(0, hh * 64),
                                skip_group_check=True)
                for nb in nbs:
                    nc.scalar.add(out=dst[:, nb * 512:(nb + 1) * 512],
                                  in_=tiles[nb], add=bias)

            for p in range(2):
                kq_group(p, range(2), 0, hq, qT2[p], qb2[p])      # Q first
                kq_group(p, range(4), 64, hb, kT2[p], kb2[p])
                kq_group(p, range(4, 8), 64, hb, kT2[p], kb2[p])

            # v in [S, dk] layout (65th column per head pre-set to ones)
            for jb in range(JB):
                ps_v = ps1v.tile([128, NH * 65], F32, name="psv", tag="psv")
                for i in range(CT):
                    nc.tensor.matmul(
                        out=ps_v, lhsT=hb[i][:, jb * 128:(jb + 1) * 128],
                        rhs=wvT[i], start=(i == 0), stop=(i == CT - 1))
                pv4 = ps_v.rearrange("p (h n) -> p h n", n=65)
                if jb % 2 == 0:
                    nc.scalar.copy(out=vS4[:, jb, :, 0:DK],
                                   in_=pv4[:, :, 0:DK])
                else:
                    nc.vector.tensor_copy(out=vS4[:, jb, :, 0:DK],
                                          in_=pv4[:, :, 0:DK])

        # ---------------- phase 2: attention --------------------------------
        resT2 = [persist.tile([128, SQ], MM_DT, name=f"res2{p}",
                              tag=f"res2{p}") for p in range(2)]
        rcp_dram = nc.dram_tensor("rcp_scratch", [2, 2 * SQ], F32)

        with tc.tile_pool(name="ps2s0", bufs=1, space="PSUM") as ps2s0, \
             tc.tile_pool(name="ps2s1", bufs=1, space="PSUM") as ps2s1, \
             tc.tile_pool(name="ps2o", bufs=1, space="PSUM") as ps2o, \
             tc.tile_pool(name="et", bufs=3) as etp, \
             tc.tile_pool(name="dn", bufs=1) as dnp:
            for p in range(2):
                po = ps2o.tile([65, 2, SQ], F32, name="po", tag="po")
                s_tiles = [None, None]
                e_tiles = [None, None]

                def scores(jb):
                    s0 = ps2s0.tile([128, SQ], F32, name="s0", tag="s0")
                    s1 = ps2s1.tile([128, SQ], F32, name="s1", tag="s1")
                    for ih in range(SQ // 512):
                        sl = slice(ih * 512, (ih + 1) * 512)
                        nc.tensor.matmul(
                            out=s0[:, sl],
                            lhsT=kT2[p][0:64, jb * 128:(jb + 1) * 128],
                            rhs=qT2[p][0:64, sl],
                            start=True, stop=True, skip_group_check=True)
                        nc.tensor.matmul(
                            out=s1[:, sl],
                            lhsT=kT2[p][64:128, jb * 128:(jb + 1) * 128],
                            rhs=qT2[p][64:128, sl],
                            start=True, stop=True, skip_group_check=True)
                    e0 = etp.tile([128, SQ], MM_DT, name="e0", tag="e0")
                    nc.scalar.activation(out=e0, in_=s0,
                                         func=mybir.ActivationFunctionType.Exp)
                    if USE_SCHRAUDOLPH:
                        e1 = etp.tile([128, SQ], MM_DT, name="e1", tag="e1")
                        nc.vector.tensor_scalar(out=e1.bitcast(I16), in0=s1,
                                                scalar1=SCH_S, scalar2=SCH_B,
                                                op0=mybir.AluOpType.mult,
                                                op1=mybir.AluOpType.add)
                    else:
                        e1 = etp.tile([128, SQ], MM_DT, name="e1", tag="e1")
                        nc.scalar.activation(
                            out=e1, in_=s1,
                            func=mybir.ActivationFunctionType.Exp)
                    e_tiles[0], e_tiles[1] = e0, e1

                def pv(jb, e0, e1):
                    for hh, e_t in ((0, e0), (1, e1)):
                        h = 2 * p + hh
                        for ih in range(SQ // 512):
                            sl = slice(ih * 512, (ih + 1) * 512)
                            nc.tensor.matmul(
                                out=po[:, hh, sl],
                                lhsT=vS[:, jb, h * 65:(h + 1) * 65],
                                rhs=e_t[:, sl],
                                start=(jb == 0), stop=(jb == JB - 1),
                                skip_group_check=True)

                prev = None
                for jb in range(JB):
                    scores(jb)
                    cur = (e_tiles[0], e_tiles[1])
                    if prev is not None:
                        pv(jb - 1, *prev)
                    prev = cur
                pv(JB - 1, *prev)

                # normalize: approx reciprocal of the denominator row (~18
                # bits, plenty for a softmax denominator; needs an SBUF src),
                # then a DMA partition-broadcast
                den = dnp.tile([1, 2, SQ], F32, name="den", tag=f"den{p}")
                nc.scalar.copy(out=den, in_=po[64:65, :, :])
                rcp = dnp.tile([1, 2, SQ], F32, name="rcp", tag=f"rcp{p}")
                nc.vector.reciprocal_approx_fast(out=rcp, in_=den)
                nc.sync.dma_start(
                    out=rcp_dram[p, :].rearrange("(o n) -> o n", o=1),
                    in_=rcp.rearrange("o a n -> o (a n)"))
                rcpb = dnp.tile([64, 2, SQ], F32, name="rcpb", tag=f"rcpb{p}")
                # 4 queue-parallel broadcast DMAs, 16 dest partitions each
                rsrc = rcp_dram[p, :]
                for qd in range(4):
                    nc.sync.dma_start(
                        out=rcpb[qd * 16:(qd + 1) * 16, :, :],
                        in_=bass.AP(tensor=rsrc.tensor, offset=rsrc.offset,
                                    ap=[[0, 16], [1, 2 * SQ]]))
                for hh in range(2):
                    nc.vector.tensor_mul(
                        out=resT2[p][hh * 64:(hh + 1) * 64, :],
                        in0=po[0:64, hh, :], in1=rcpb[:, hh, :])

        # ---------------- phase 3: out-projection + residual ----------------
        with tc.tile_pool(name="ps3", bufs=1, space="PSUM") as ps3, \
             tc.tile_pool(name="ob", bufs=2) as obp:
            # fold the v-bias through out_w: wbv[c] = sum_hd out_w[c,hd]*bv[hd]
            ps_wbv = [ps3.tile([128, 1], F32, name=f"wbv{t2}", tag=f"wbv{t2}")
                      for t2 in range(CT)]
            for ct_i in range(CT):
                for p in range(2):
                    nc.tensor.matmul(
                        out=ps_wbv[ct_i],
                        lhsT=owT2[p][:, ct_i * 128:(ct_i + 1) * 128],
                        rhs=bv2[p], start=(p == 0), stop=(p == 1),
                        skip_group_check=True)
            outb2 = [obp.tile([128, 1], F32, name=f"ob2{t2}", tag=f"ob2{t2}")
                     for t2 in range(CT)]
            for ct_i in range(CT):
                nc.vector.tensor_add(out=outb2[ct_i], in0=outb[ct_i],
                                     in1=ps_wbv[ct_i])

            ps_outs = [ps3.tile([128, SQ], F32, name=f"pso3{t2}", tag="pso3")
                       for t2 in range(CT)]
            for p in range(2):
                for ct_i in range(CT):
                    for ih in range(SQ // 512):
                        sl = slice(ih * 512, (ih + 1) * 512)
                        nc.tensor.matmul(
                            out=ps_outs[ct_i][:, sl],
                            lhsT=owT2[p][:, ct_i * 128:(ct_i + 1) * 128],
                            rhs=resT2[p][:, sl],
                            start=(p == 0), stop=(p == 1),
                            skip_group_check=True)
            for ct_i in range(CT):
                obuf = obp.tile([128, SQ], F32, name="obuf", tag="obuf")
                nc.vector.scalar_tensor_tensor(
                    out=obuf, in0=ps_outs[ct_i], scalar=outb2[ct_i],
                    in1=xqt[ct_i],
                    op0=mybir.AluOpType.add, op1=mybir.AluOpType.add)
                for oc in range(4):
                    sl = slice(oc * 256, (oc + 1) * 256)
                    nc.sync.dma_start(
                        out=out[ct_i * 128:(ct_i + 1) * 128, sl],
                        in_=obuf[:, sl])


_NC_CACHE = None


def _get_nc():
    global _NC_CACHE
    if _NC_CACHE is None:
        _NC_CACHE = build_nc()
    return _NC_CACHE


def _make_in_maps(x, gn_w, gn_b, proj_w, proj_b, out_w, out_b):
    xf = np.ascontiguousarray(np.asarray(x, dtype=np.float32)).reshape(B, C, S)
    shared = {
        "proj_w": np.ascontiguousarray(proj_w, dtype=np.float32),
        "proj_b": np.ascontiguousarray(proj_b, dtype=np.float32),
        "out_w": np.ascontiguousarray(out_w, dtype=np.float32),
        "out_b": np.ascontiguousarray(out_b, dtype=np.float32),
        "gn_w": np.ascontiguousarray(gn_w, dtype=np.float32),
        "gn_b": np.ascontiguousarray(gn_b, dtype=np.float32),
    }
    in_maps = []
    for core in range(N_CORES):
        b, chunk = core // CHUNKS, core % CHUNKS
        in_maps.append({
            "x": np.ascontiguousarray(xf[b]),
            "xq": np.ascontiguousarray(xf[b][:, chunk * SQ:(chunk + 1) * SQ]),
            **shared,
        })
    return in_maps


def _gather(results):
    outp = np.empty((B, C, S), dtype=np.float32)
    for core in range(N_CORES):
        b, chunk = core // CHUNKS, core % CHUNKS
        outp[b][:, chunk * SQ:(chunk + 1) * SQ] = results[core]["out"]
    return outp.reshape(B, C, H, W)


def kernel(x, gn_w, gn_b, proj_w, proj_b, out_w, out_b):
    import concourse.bass_utils as bu
    bu.upload_artifacts = lambda tmpdir: tmpdir  # no artifact bucket in sandbox

    in_maps = _make_in_maps(x, gn_w, gn_b, proj_w, proj_b, out_w, out_b)
    res = bu.run_bass_kernel_spmd(_get_nc(), in_maps, list(range(N_CORES)))
    return _gather(res.results)


# revision 25
# speedup vs baseline: 1.3678x; 1.0980x over previous
"""Trainium2 Bass kernel for nn_AttentionBlock (GroupNorm + 4-head self-attention
over S=4096 + output projection + residual) on x:[2, 256, 64, 64].

Sharding: 8 cores = (batch 2) x (query-chunk 4). Each core receives the full
image of its batch (for GroupNorm stats and K/V over all 4096 positions) plus
its own 1024-query chunk, and produces the exact [256, 1024] output slice.
No cross-core collectives.

v2 structure (per core):
  phase 0: chunked x DMA with bn_stats pipelined right behind each chunk;
           PE does weight transposes + the group-aggregation matmuls early.
           Per-channel scale/shift -> hb (full image) / hq (query chunk), bf16.
  phase 1: K/Q projections per head-pair (col-tiled, PSUM), V projection per
           128-key block; drains split across ScalarE / VectorE.
  phase 2: attention per head-pair. The two heads' score matmuls (K=dk=64)
           run CONCURRENTLY as 64x128 row-tiles (head A on PE rows 0:64,
           head B on rows 64:128 - the kT2/qT2 pair packing puts each head's
           operands on exactly those SBUF partitions). exp splits across
           engines: head A on ScalarE (native EXP), head B on VectorE via a
           Schraudolph bit-trick (y = int16(s*184.665+16250.5) bitcast bf16,
           +-3% per-weight; the softmax ratio cancels the systematic part).
           PV accumulates [65, 2, SQ] in one 4-bank PSUM tile (ones column
           -> denominators). PV(jb-1) is issued between S(jb) and S(jb+1) so
           the PE stays busy during the exp drain.
  phase 3: per pair: reciprocal of the denominator row, DMA partition-
           broadcast (SBUF->SBUF, no DRAM round trip), normalize into
           resT2[p] [128, SQ] (head pair stacked). Out-projection contracts
           both heads of a pair in single K=128 matmuls (owT2 pair-packed),
           accumulating both pairs into one PSUM group; + out_b (with the
           v-bias folded through out_w) + residual, DMA out.
"""

import contextlib
import math

import numpy as np

import concourse.bass as bass
import concourse.tile as tile
from concourse import mybir
from concourse.bacc import Bacc
from concourse.masks import make_identity

# Problem constants (hardcoded per harness contract).
B = 2
C = 256
H = W = 64
S = H * W            # 4096
NH = 4
DK = 64
EPS = 1e-5
SCALE2 = 0.125       # (1/sqrt(sqrt(dk)))^2, folded into Wq/bq
N_CORES = 8
CHUNKS = N_CORES // B    # query chunks per batch
SQ = S // CHUNKS         # queries per core (1024)
JB = S // 128            # 32 key blocks
CT = C // 128            # 2 channel tiles

F32 = mybir.dt.float32
BF16 = mybir.dt.bfloat16
I16 = mybir.dt.int16
I8 = mybir.dt.int8
FP8V = mybir.dt.float8e4     # V values (3-bit mantissa, |v| << 240)
FP8E = mybir.dt.float8e5     # exp(score) values (range to 57344, no clamp)
MM_DT = BF16

# Schraudolph exp-as-float-bits: bits = round(s*S + B), per target dtype
_r_max = 1.4426950408889634 / 2 ** 0.44269504088896340
_corr = math.log2(_r_max) / 2.0
SCH_S = 128.0 * math.log2(math.e)             # bf16 (7 mantissa bits)
SCH_B = 128.0 * (127.0 - _corr)
SCH8_S = 4.0 * math.log2(math.e)              # fp8e5m2 (2 mantissa bits)
SCH8_B = 4.0 * (15.0 - _corr)

USE_SCHRAUDOLPH = True   # h1 exp on DVE via int bit trick (else ACT exp)
USE_PV_DR = True         # PV matmuls in fp8 DoubleRow (2 key blocks / MM)


def build_nc():
    nc = Bacc()
    x = nc.declare_dram_parameter("x", [C, S], F32, isOutput=False)
    xq = nc.declare_dram_parameter("xq", [C, SQ], F32, isOutput=False)
    proj_w = nc.declare_dram_parameter("proj_w", [3 * C, C], F32, isOutput=False)
    proj_b = nc.declare_dram_parameter("proj_b", [3 * C], F32, isOutput=False)
    out_w = nc.declare_dram_parameter("out_w", [C, C], F32, isOutput=False)
    out_b = nc.declare_dram_parameter("out_b", [C], F32, isOutput=False)
    gn_w = nc.declare_dram_parameter("gn_w", [C], F32, isOutput=False)
    gn_b = nc.declare_dram_parameter("gn_b", [C], F32, isOutput=False)
    out = nc.declare_dram_parameter("out", [C, SQ], F32, isOutput=True)

    with tile.TileContext(nc) as tc:
        _emit(nc, tc, x, xq, proj_w, proj_b, out_w, out_b, gn_w, gn_b, out)
    nc.finalize()
    return nc


def _emit(nc, tc, x, xq, proj_w, proj_b, out_w, out_b, gn_w, gn_b, out):
    with contextlib.ExitStack() as ctx:
        const = ctx.enter_context(tc.tile_pool(name="const", bufs=1))
        persist = ctx.enter_context(tc.tile_pool(name="persist", bufs=1))

        ident = const.tile([128, 128], F32)
        make_identity(nc, ident)

        # ---------------- phase 0: loads + stats (chunk-pipelined) ----------
        # weight DMAs go first (small; transposes are early PE work), then
        # the 5MB of activations
        XCH = 4                  # x DMA chunks per tile
        CW = S // XCH            # 1024 columns per chunk

        def load_col(dram_vec, lo, n, tag):
            col = const.tile([n, 1], F32, tag=tag)
            nc.gpsimd.dma_start(
                out=col, in_=dram_vec[lo:lo + n].rearrange("(p o) -> p o", o=1)
            )
            return col

        gnw = [load_col(gn_w, t * 128, 128, f"gnw{t}") for t in range(CT)]
        gnb = [load_col(gn_b, t * 128, 128, f"gnb{t}") for t in range(CT)]
        outb = [load_col(out_b, t * 128, 128, f"outb{t}") for t in range(CT)]
        # pair-stacked q/k biases [128, 1] (q pre-scaled by 1/8)
        qb2, kb2 = [], []
        for p in range(2):
            qbp = const.tile([128, 1], F32, name=f"qb2{p}", tag=f"qb2{p}")
            kbp = const.tile([128, 1], F32, name=f"kb2{p}", tag=f"kb2{p}")
            for hh in range(2):
                h = 2 * p + hh
                nc.sync.dma_start(
                    out=qbp[hh * 64:(hh + 1) * 64, :],
                    in_=proj_b[h * 192:h * 192 + 64]
                    .rearrange("(p o) -> p o", o=1))
                nc.sync.dma_start(
                    out=kbp[hh * 64:(hh + 1) * 64, :],
                    in_=proj_b[h * 192 + 64:h * 192 + 128]
                    .rearrange("(p o) -> p o", o=1))
            nc.vector.tensor_scalar_mul(out=qbp, in0=qbp, scalar1=SCALE2)
            qb2.append(qbp)
            kb2.append(kbp)
        # v bias pair-stacked [128, 1] for the out_w fold (bf16 for matmul)
        bv2 = []
        for p in range(2):
            bvp = const.tile([128, 1], F32, name=f"bv2{p}", tag=f"bv2{p}")
            for hh in range(2):
                h = 2 * p + hh
                nc.sync.dma_start(
                    out=bvp[hh * 64:(hh + 1) * 64, :],
                    in_=proj_b[h * 192 + 128:h * 192 + 192]
                    .rearrange("(p o) -> p o", o=1))
            bvb = const.tile([128, 1], MM_DT, name=f"bv2b{p}", tag=f"bv2b{p}")
            nc.vector.tensor_copy(out=bvb, in_=bvp)
            bv2.append(bvb)

        with tc.tile_pool(name="ph0", bufs=2) as ph0, \
             tc.tile_pool(name="pw0", bufs=3) as pw0, \
             tc.tile_pool(name="ps0", bufs=4, space="PSUM") as ps0:

            # ---- weight loads + PE transposes (early PE work) ----
            # proj_w^T: pwT[ct][c_local, r] = proj_w[r, ct*128+c_local]
            pwT = [persist.tile([128, 3 * C], MM_DT, name=f"pwT{i}",
                                tag=f"pwT{i}") for i in range(CT)]
            for r in range(6):
                pw_r = pw0.tile([128, C], F32, name="pw", tag="pw")
                nc.sync.dma_start(out=pw_r, in_=proj_w[r * 128:(r + 1) * 128, :])
                for ct_i in range(CT):
                    pst = ps0.tile([128, 128], F32, name="tr", tag="ps0t")
                    nc.tensor.transpose(
                        out=pst, in_=pw_r[:, ct_i * 128:(ct_i + 1) * 128],
                        identity=ident)
                    nc.scalar.copy(out=pwT[ct_i][:, r * 128:(r + 1) * 128],
                                   in_=pst)
            # fold 1/8 into the q columns
            for ct_i in range(CT):
                qcols = pwT[ct_i].rearrange("p (h n) -> p h n", n=192)[:, :, 0:DK]
                nc.vector.tensor_scalar_mul(out=qcols, in0=qcols, scalar1=SCALE2)

            # v columns re-packed with a zero 65th column per head
            wvT = []
            for ct_i in range(CT):
                wv = persist.tile([128, NH * 65], MM_DT, name=f"wvT{ct_i}",
                                  tag=f"wvT{ct_i}")
                nc.gpsimd.memset(wv, 0.0)
                nc.vector.tensor_copy(
                    out=wv.rearrange("p (h n) -> p h n", n=65)[:, :, 0:DK],
                    in_=pwT[ct_i].rearrange("p (h n) -> p h n", n=192)
                    [:, :, 128:192],
                )
                wvT.append(wv)

            # out_w^T pair-packed: owT2[p][hh*64+d, c] = out_w[c, (2p+hh)*64+d]
            owT2 = [persist.tile([128, C], MM_DT, name=f"owT2{p}",
                                 tag=f"owT2{p}") for p in range(2)]
            for ct_i in range(CT):
                ow_c = ph0.tile([128, C], F32, name="ow", tag="ow")
                nc.sync.dma_start(out=ow_c,
                                  in_=out_w[ct_i * 128:(ct_i + 1) * 128, :])
                for h in range(NH):
                    p, hh = h // 2, h % 2
                    pst = ps0.tile([64, 128], F32, name="trh", tag="ps0t")
                    nc.tensor.transpose(
                        out=pst, in_=ow_c[:, h * 64:(h + 1) * 64],
                        identity=ident)
                    nc.scalar.copy(
                        out=owT2[p][hh * 64:(hh + 1) * 64,
                                    ct_i * 128:(ct_i + 1) * 128],
                        in_=pst)

            # ---- activation DMAs (after the weights are queued) ----
            xt = []
            for t in range(CT):
                xt_t = persist.tile([128, S], F32, name=f"xt{t}", tag=f"xt{t}")
                xt.append(xt_t)
            for c in range(XCH):
                for t in range(CT):
                    nc.sync.dma_start(
                        out=xt[t][:, c * CW:(c + 1) * CW],
                        in_=x[t * 128:(t + 1) * 128, c * CW:(c + 1) * CW])
            xqt = []
            for t in range(CT):
                xq_t = persist.tile([128, SQ], F32, name=f"xq{t}",
                                    tag=f"xq{t}")
                nc.sync.dma_start(out=xq_t, in_=xq[t * 128:(t + 1) * 128, :])
                xqt.append(xq_t)

            # ---- groupnorm stats (chunk-pipelined behind the x DMA) ----
            hb, hq = [], []
            scale_t, shift_t = [], []
            for t in range(CT):
                st6 = ph0.tile([128, 8, 6], F32, name=f"st6{t}", tag=f"st6{t}")
                xv = xt[t].rearrange("p (n f) -> p n f", f=512)
                for i in range(8):
                    nc.vector.bn_stats(out=st6[:, i, :], in_=xv[:, i, :])
                mv = ph0.tile([128, 2], F32, name="mv", tag=f"mv{t}")
                nc.vector.bn_aggr(out=mv, in_=st6)
                st2 = ph0.tile([128, 2], F32, name="st2", tag=f"st2{t}")
                sq = ph0.tile([128, 1], F32, name="sq", tag=f"sq{t}")
                nc.vector.tensor_mul(out=sq, in0=mv[:, 0:1], in1=mv[:, 0:1])
                nc.vector.tensor_copy(out=st2[:, 0:1], in_=mv[:, 0:1])
                nc.vector.tensor_add(out=st2[:, 1:2], in0=sq, in1=mv[:, 1:2])

                # group aggregation via one-hot(1/8) matmul -> [16, 2]
                gmat = ph0.tile([128, 16], F32, name="gmat", tag=f"gmat{t}")
                nc.gpsimd.memset(gmat, 0.125)
                nc.gpsimd.affine_select(
                    out=gmat, in_=gmat, compare_op=mybir.AluOpType.is_ge,
                    fill=0.0, base=0, pattern=[[-8, 16]], channel_multiplier=1)
                nc.gpsimd.affine_select(
                    out=gmat, in_=gmat, compare_op=mybir.AluOpType.is_ge,
                    fill=0.0, base=7, pattern=[[8, 16]], channel_multiplier=-1)
                ps_g = ps0.tile([16, 2], F32, name="psg", tag="ps0t")
                nc.tensor.matmul(out=ps_g, lhsT=gmat, rhs=st2,
                                 start=True, stop=True)
                gs = ph0.tile([16, 2], F32, name="gs", tag=f"gs{t}")
                nc.vector.tensor_copy(out=gs, in_=ps_g)

                # var_g = E[x^2]-mean^2; rstd = 1/sqrt(var+eps) + Newton polish
                sqg = ph0.tile([16, 1], F32, name="sqg", tag=f"sqg{t}")
                varg = ph0.tile([16, 1], F32, name="varg", tag=f"varg{t}")
                nc.vector.tensor_mul(out=sqg, in0=gs[:, 0:1], in1=gs[:, 0:1])
                nc.vector.tensor_sub(out=varg, in0=gs[:, 1:2], in1=sqg)
                epst = ph0.tile([16, 1], F32, name="epst", tag=f"epst{t}")
                nc.vector.memset(epst, EPS)
                srt = ph0.tile([16, 1], F32, name="srt", tag=f"srt{t}")
                nc.scalar.activation(out=srt, in_=varg,
                                     func=mybir.ActivationFunctionType.Sqrt,
                                     bias=epst, scale=1.0)
                r0 = ph0.tile([16, 1], F32, name="r0", tag=f"r0{t}")
                nc.vector.reciprocal(out=r0, in_=srt)
                ve = ph0.tile([16, 1], F32, name="ve", tag=f"ve{t}")
                nc.vector.tensor_scalar_add(out=ve, in0=varg, scalar1=EPS)
                r2 = ph0.tile([16, 1], F32, name="r2", tag=f"r2{t}")
                nc.vector.tensor_mul(out=r2, in0=r0, in1=r0)
                t1 = ph0.tile([16, 1], F32, name="t1", tag=f"t1{t}")
                nc.vector.tensor_mul(out=t1, in0=ve, in1=r2)
                t2 = ph0.tile([16, 1], F32, name="t2", tag=f"t2{t}")
                nc.vector.tensor_scalar(out=t2, in0=t1, scalar1=-0.5,
                                        scalar2=1.5,
                                        op0=mybir.AluOpType.mult,
                                        op1=mybir.AluOpType.add)
                rstd = ph0.tile([16, 1], F32, name="rstd", tag=f"rstd{t}")
                nc.vector.tensor_mul(out=rstd, in0=r0, in1=t2)

                # broadcast group params to channels with G^T one-hot matmul
                ps_gt = ps0.tile([16, 128], F32, name="psgt", tag="ps0t")
                nc.tensor.transpose(out=ps_gt, in_=gmat, identity=ident)
                g2 = ph0.tile([16, 128], F32, name="g2", tag=f"g2{t}")
                nc.scalar.mul(out=g2, in_=ps_gt, mul=8.0)
                grp2 = ph0.tile([16, 2], F32, name="grp2", tag=f"grp2{t}")
                nc.vector.tensor_copy(out=grp2[:, 0:1], in_=gs[:, 0:1])
                nc.vector.tensor_copy(out=grp2[:, 1:2], in_=rstd)
                ps_b = ps0.tile([128, 2], F32, name="psb", tag="ps0t")
                nc.tensor.matmul(out=ps_b, lhsT=g2, rhs=grp2,
                                 start=True, stop=True)
                chst = ph0.tile([128, 2], F32, name="chst", tag=f"chst{t}")
                nc.vector.tensor_copy(out=chst, in_=ps_b)

                # per-channel scale/shift with gamma/beta folded in
                scale = ph0.tile([128, 1], F32, name="scale", tag=f"scale{t}")
                nc.vector.tensor_mul(out=scale, in0=chst[:, 1:2], in1=gnw[t])
                tmp2 = ph0.tile([128, 1], F32, name="tmp2", tag=f"tmp2{t}")
                nc.vector.tensor_mul(out=tmp2, in0=chst[:, 0:1], in1=scale)
                shift = ph0.tile([128, 1], F32, name="shift", tag=f"shift{t}")
                nc.vector.tensor_sub(out=shift, in0=gnb[t], in1=tmp2)
                scale_t.append(scale)
                shift_t.append(shift)

                hb.append(persist.tile([128, S], MM_DT, name=f"hb{t}",
                                       tag=f"hb{t}"))
                hq.append(persist.tile([128, SQ], MM_DT, name=f"hq{t}",
                                       tag=f"hq{t}"))

            # normalized activations; hq first (the Q projection is first
            # consumer), then hb tile-interleaved per chunk. Tile 0 on
            # VectorE, tile 1 on ScalarE (Identity with per-channel
            # scale/bias) so the chunks land ~2x faster.
            def norm_chunk(t, dst, src):
                if t == 0:
                    nc.vector.tensor_scalar(out=dst, in0=src,
                                            scalar1=scale_t[t],
                                            scalar2=shift_t[t],
                                            op0=mybir.AluOpType.mult,
                                            op1=mybir.AluOpType.add)
                else:
                    nc.scalar.activation(
                        out=dst, in_=src,
                        func=mybir.ActivationFunctionType.Identity,
                        bias=shift_t[t], scale=scale_t[t])

            for t in range(CT):
                norm_chunk(t, hq[t], xqt[t])
            for c in range(XCH):
                for t in range(CT):
                    sl = slice(c * CW, (c + 1) * CW)
                    norm_chunk(t, hb[t][:, sl], xt[t][:, sl])

        # ---------------- phase 1: projections ------------------------------
        kT2 = [persist.tile([128, S], MM_DT, name=f"kT2{p}", tag=f"kT2{p}")
               for p in range(2)]
        qT2 = [persist.tile([128, SQ], MM_DT, name=f"qT2{p}", tag=f"qT2{p}")
               for p in range(2)]
        if USE_PV_DR:
            # key-block pairs for DoubleRow: [p, jb-pair, jb%2, head, 68]
            # (68 = 65 + pad so the Ko-plane stride 4*68=272 is 16B aligned)
            vS = persist.tile([128, JB // 2, 2, NH, 68], FP8V, name="vS")
            nc.gpsimd.memset(vS[:, :, :, :, 64:65], 1.0)
        else:
            vS = persist.tile([128, JB, NH * 65], MM_DT, name="vS")
            vS4 = vS.rearrange("p j (h n) -> p j h n", n=65)
            nc.gpsimd.memset(vS4[:, :, :, 64:65], 1.0)

        with tc.tile_pool(name="ps1k", bufs=4, space="PSUM") as ps1k, \
             tc.tile_pool(name="ps1v", bufs=3, space="PSUM") as ps1v:

            def kq_group(p, nbs, wsel, src, dst, bias):
                # weight-resident sweep: psum tiles for all nbs held across
                # the (ct, hh) weight loads -> 16 dense back-to-back matmuls
                tiles = {nb: ps1k.tile([128, 512], F32, name="psk", tag="pskq")
                         for nb in nbs}
                for i in range(CT):
                    for hh in range(2):
                        h = 2 * p + hh
                        w = pwT[i][:, h * 192 + wsel:h * 192 + wsel + 64]
                        for nb in nbs:
                            nc.tensor.matmul(
                                out=tiles[nb][hh * 64:(hh + 1) * 64, :],
                                lhsT=w,
                                rhs=src[i][:, nb * 512:(nb + 1) * 512],
                                start=(i == 0), stop=(i == CT - 1),
                                tile_position=(0, hh * 64),
                                skip_group_check=True)
                for nb in nbs:
                    nc.scalar.add(out=dst[:, nb * 512:(nb + 1) * 512],
                                  in_=tiles[nb], add=bias)

            for p in range(2):
                kq_group(p, range(2), 0, hq, qT2[p], qb2[p])      # Q first
                kq_group(p, range(4), 64, hb, kT2[p], kb2[p])
                kq_group(p, range(4, 8), 64, hb, kT2[p], kb2[p])

            # v in [S, dk] layout (65th column per head pre-set to ones)
            for jb in range(JB):
                ps_v = ps1v.tile([128, NH * 65], F32, name="psv", tag="psv")
                for i in range(CT):
                    nc.tensor.matmul(
                        out=ps_v, lhsT=hb[i][:, jb * 128:(jb + 1) * 128],
                        rhs=wvT[i], start=(i == 0), stop=(i == CT - 1))
                pv4 = ps_v.rearrange("p (h n) -> p h n", n=65)
                if USE_PV_DR:
                    vdst = vS[:, jb // 2, jb % 2, :, 0:DK]
                else:
                    vdst = vS4[:, jb, :, 0:DK]
                if jb % 2 == 0:
                    nc.scalar.copy(out=vdst, in_=pv4[:, :, 0:DK])
                else:
                    nc.vector.tensor_copy(out=vdst, in_=pv4[:, :, 0:DK])

        # ---------------- phase 2: attention --------------------------------
        resT2 = [persist.tile([128, SQ], MM_DT, name=f"res2{p}",
                              tag=f"res2{p}") for p in range(2)]
        rcp_dram = nc.dram_tensor("rcp_scratch", [2, 2 * SQ], F32)

        with tc.tile_pool(name="ps2s0", bufs=1, space="PSUM") as ps2s0, \
             tc.tile_pool(name="ps2s1", bufs=1, space="PSUM") as ps2s1, \
             tc.tile_pool(name="ps2o", bufs=1, space="PSUM") as ps2o, \
             tc.tile_pool(name="et", bufs=3) as etp, \
             tc.tile_pool(name="dn", bufs=1) as dnp:
            for p in range(2):
                po = ps2o.tile([65, 2, SQ], F32, name="po", tag="po")

                def scores_mm(jb):
                    s0 = ps2s0.tile([128, SQ], F32, name="s0", tag="s0")
                    s1 = ps2s1.tile([128, SQ], F32, name="s1", tag="s1")
                    for ih in range(SQ // 512):
                        sl = slice(ih * 512, (ih + 1) * 512)
                        nc.tensor.matmul(
                            out=s0[:, sl],
                            lhsT=kT2[p][0:64, jb * 128:(jb + 1) * 128],
                            rhs=qT2[p][0:64, sl],
                            start=True, stop=True, skip_group_check=True)
                        nc.tensor.matmul(
                            out=s1[:, sl],
                            lhsT=kT2[p][64:128, jb * 128:(jb + 1) * 128],
                            rhs=qT2[p][64:128, sl],
                            start=True, stop=True, skip_group_check=True)
                    return s0, s1

                if USE_PV_DR:
                    def exps(s0, s1, pair, k):
                        e0, e1 = pair
                        nc.scalar.activation(
                            out=e0[:, k, :], in_=s0,
                            func=mybir.ActivationFunctionType.Exp)
                        if USE_SCHRAUDOLPH:
                            nc.vector.tensor_scalar(
                                out=e1.bitcast(I8)[:, k, :], in0=s1,
                                scalar1=SCH8_S, scalar2=SCH8_B,
                                op0=mybir.AluOpType.mult,
                                op1=mybir.AluOpType.add)
                        else:
                            nc.scalar.activation(
                                out=e1[:, k, :], in_=s1,
                                func=mybir.ActivationFunctionType.Exp)

                    def pv_dr(jbp, e0, e1):
                        for hh, e_t in ((0, e0), (1, e1)):
                            h = 2 * p + hh
                            for ih in range(SQ // 512):
                                sl = slice(ih * 512, (ih + 1) * 512)
                                nc.tensor.matmul(
                                    out=po[:, hh, sl],
                                    lhsT=vS[:, jbp, :, h, 0:65],
                                    rhs=e_t[:, :, sl],
                                    start=(jbp == 0),
                                    stop=(jbp == JB // 2 - 1),
                                    perf_mode=mybir.MatmulPerfMode.DoubleRow,
                                    skip_group_check=True)

                    prev_pair = cur_pair = None
                    for jb in range(JB):
                        s0, s1 = scores_mm(jb)
                        if jb % 2 == 0:
                            cur_pair = (
                                etp.tile([128, 2, SQ], FP8E, name="e0",
                                         tag="e0"),
                                etp.tile([128, 2, SQ], FP8E, name="e1",
                                         tag="e1"))
                        exps(s0, s1, cur_pair, jb % 2)
                        if jb % 2 == 1:
                            if prev_pair is not None:
                                pv_dr(jb // 2 - 1, *prev_pair)
                            prev_pair = cur_pair
                    pv_dr(JB // 2 - 1, *prev_pair)
                else:
                    def exps_bf(s0, s1):
                        e0 = etp.tile([128, SQ], MM_DT, name="e0", tag="e0")
                        nc.scalar.activation(
                            out=e0, in_=s0,
                            func=mybir.ActivationFunctionType.Exp)
                        e1 = etp.tile([128, SQ], MM_DT, name="e1", tag="e1")
                        if USE_SCHRAUDOLPH:
                            nc.vector.tensor_scalar(
                                out=e1.bitcast(I16), in0=s1,
                                scalar1=SCH_S, scalar2=SCH_B,
                                op0=mybir.AluOpType.mult,
                                op1=mybir.AluOpType.add)
                        else:
                            nc.scalar.activation(
                                out=e1, in_=s1,
                                func=mybir.ActivationFunctionType.Exp)
                        return e0, e1

                    def pv(jb, e0, e1):
                        vS4 = vS.rearrange("p j (h n) -> p j h n", n=65)
                        for hh, e_t in ((0, e0), (1, e1)):
                            h = 2 * p + hh
                            for ih in range(SQ // 512):
                                sl = slice(ih * 512, (ih + 1) * 512)
                                nc.tensor.matmul(
                                    out=po[:, hh, sl],
                                    lhsT=vS4[:, jb, h, 0:65],
                                    rhs=e_t[:, sl],
                                    start=(jb == 0), stop=(jb == JB - 1),
                                    skip_group_check=True)

                    prev = None
                    for jb in range(JB):
                        s0, s1 = scores_mm(jb)
                        cur = exps_bf(s0, s1)
                        if prev is not None:
                            pv(jb - 1, *prev)
                        prev = cur
                    pv(JB - 1, *prev)

                # normalize: approx reciprocal of the denominator row (~18
                # bits, plenty for a softmax denominator; needs an SBUF src),
                # then a DMA partition-broadcast
                den = dnp.tile([1, 2, SQ], F32, name="den", tag=f"den{p}")
                nc.scalar.copy(out=den, in_=po[64:65, :, :])
                rcp = dnp.tile([1, 2, SQ], F32, name="rcp", tag=f"rcp{p}")
                nc.vector.reciprocal_approx_fast(out=rcp, in_=den)
                nc.sync.dma_start(
                    out=rcp_dram[p, :].rearrange("(o n) -> o n", o=1),
                    in_=rcp.rearrange("o a n -> o (a n)"))
                rcpb = dnp.tile([64, 2, SQ], F32, name="rcpb", tag=f"rcpb{p}")
                # 4 queue-parallel broadcast DMAs, 16 dest partitions each
                rsrc = rcp_dram[p, :]
                for qd in range(4):
                    nc.sync.dma_start(
                        out=rcpb[qd * 16:(qd + 1) * 16, :, :],
                        in_=bass.AP(tensor=rsrc.tensor, offset=rsrc.offset,
                                    ap=[[0, 16], [1, 2 * SQ]]))
                for hh in range(2):
                    nc.vector.tensor_mul(
                        out=resT2[p][hh * 64:(hh + 1) * 64, :],
                        in0=po[0:64, hh, :], in1=rcpb[:, hh, :])

        # ---------------- phase 3: out-projection + residual ----------------
        with tc.tile_pool(name="ps3", bufs=1, space="PSUM") as ps3, \
             tc.tile_pool(name="ob", bufs=2) as obp:
            # fold the v-bias through out_w: wbv[c] = sum_hd out_w[c,hd]*bv[hd]
            ps_wbv = [ps3.tile([128, 1], F32, name=f"wbv{t2}", tag=f"wbv{t2}")
                      for t2 in range(CT)]
            for ct_i in range(CT):
                for p in range(2):
                    nc.tensor.matmul(
                        out=ps_wbv[ct_i],
                        lhsT=owT2[p][:, ct_i * 128:(ct_i + 1) * 128],
                        rhs=bv2[p], start=(p == 0), stop=(p == 1),
                        skip_group_check=True)
            outb2 = [obp.tile([128, 1], F32, name=f"ob2{t2}", tag=f"ob2{t2}")
                     for t2 in range(CT)]
            for ct_i in range(CT):
                nc.vector.tensor_add(out=outb2[ct_i], in0=outb[ct_i],
                                     in1=ps_wbv[ct_i])

            ps_outs = [ps3.tile([128, SQ], F32, name=f"pso3{t2}", tag="pso3")
                       for t2 in range(CT)]
            for p in range(2):
                for ct_i in range(CT):
                    for ih in range(SQ // 512):
                        sl = slice(ih * 512, (ih + 1) * 512)
                        nc.tensor.matmul(
                            out=ps_outs[ct_i][:, sl],
                            lhsT=owT2[p][:, ct_i * 128:(ct_i + 1) * 128],
                            rhs=resT2[p][:, sl],
                            start=(p == 0), stop=(p == 1),
                            skip_group_check=True)
            for ct_i in range(CT):
                obuf = obp.tile([128, SQ], F32, name="obuf", tag="obuf")
                nc.vector.scalar_tensor_tensor(
                    out=obuf, in0=ps_outs[ct_i], scalar=outb2[ct_i],
                    in1=xqt[ct_i],
                    op0=mybir.AluOpType.add, op1=mybir.AluOpType.add)
                for oc in range(4):
                    sl = slice(oc * 256, (oc + 1) * 256)
                    nc.sync.dma_start(
                        out=out[ct_i * 128:(ct_i + 1) * 128, sl],
                        in_=obuf[:, sl])


_NC_CACHE = None


def _get_nc():
    global _NC_CACHE
    if _NC_CACHE is None:
        _NC_CACHE = build_nc()
    return _NC_CACHE


def _make_in_maps(x, gn_w, gn_b, proj_w, proj_b, out_w, out_b):
    xf = np.ascontiguousarray(np.asarray(x, dtype=np.float32)).reshape(B, C, S)
    shared = {
        "proj_w": np.ascontiguousarray(proj_w, dtype=np.float32),
        "proj_b": np.ascontiguousarray(proj_b, dtype=np.float32),
        "out_w": np.ascontiguousarray(out_w, dtype=np.float32),
        "out_b": np.ascontiguousarray(out_b, dtype=np.float32),
        "gn_w": np.ascontiguousarray(gn_w, dtype=np.float32),
        "gn_b": np.ascontiguousarray(gn_b, dtype=np.float32),
    }
    in_maps = []
    for core in range(N_CORES):
        b, chunk = core // CHUNKS, core % CHUNKS
        in_maps.append({
            "x": np.ascontiguousarray(xf[b]),
            "xq": np.ascontiguousarray(xf[b][:, chunk * SQ:(chunk + 1) * SQ]),
            **shared,
        })
    return in_maps


def _gather(results):
    outp = np.empty((B, C, S), dtype=np.float32)
    for core in range(N_CORES):
        b, chunk = core // CHUNKS, core % CHUNKS
        outp[b][:, chunk * SQ:(chunk + 1) * SQ] = results[core]["out"]
    return outp.reshape(B, C, H, W)


def kernel(x, gn_w, gn_b, proj_w, proj_b, out_w, out_b):
    import concourse.bass_utils as bu
    bu.upload_artifacts = lambda tmpdir: tmpdir  # no artifact bucket in sandbox

    in_maps = _make_in_maps(x, gn_w, gn_b, proj_w, proj_b, out_w, out_b)
    res = bu.run_bass_kernel_spmd(_get_nc(), in_maps, list(range(N_CORES)))
    return _gather(res.results)
